# revision 1
# baseline (speedup 1.0000x reference)
"""BasicTransformerBlock on 8 TRN2 NeuronCores.

Sharding: data-parallel, core = (batch b in 0..3) x (sequence half h in 0..1).
Each core receives its batch element's full sequence rotated so its local 512
rows come first (softmax over keys is permutation invariant), computes K/V of
attn1 for all 1024 tokens (duplicated across the pair, ~10% extra FLOPs, zero
collectives), and everything else for its 512 local tokens only.

On-chip layout: feature-major activations [features on partitions, tokens on
free axis] so every projection consumes natural-layout weights as the matmul
stationary operand. Matmuls run in fp16 (weights pre-cast on host); the
residual stream, LN math and PSUM accumulation stay fp32. LayerNorm partition
reductions and per-token broadcasts use float32r ones-matmuls. Attention
softmax denominators come free from a ones-column appended to V.
"""

import sys
import types

sys.path.insert(0, "/opt/trn_rl_repo")

# concourse fetches the NTFF profile hook from antenv.axon_hooks, which the
# agent image's antenv stub lacks. Register a shim so trace=True works.
if "antenv.axon_hooks" not in sys.modules:
    _hooks = types.ModuleType("antenv.axon_hooks")
    _HOOK = [None]

    def _get_hook():
        if _HOOK[0] is None:
            try:
                from trn_agent_boot.trn_boot import _ntff_profile_via_ctypes

                _HOOK[0] = _ntff_profile_via_ctypes("/opt/axon/libaxon_pjrt.so")
            except Exception:
                _HOOK[0] = None
        return _HOOK[0]

    _hooks.get_axon_ntff_profile_hook = _get_hook
    _hooks.set_axon_ntff_profile_hook = lambda h: _HOOK.__setitem__(0, h)
    sys.modules["antenv.axon_hooks"] = _hooks
    try:
        import antenv

        antenv.axon_hooks = _hooks
    except ImportError:
        pass

import numpy as np

import concourse.bass as bass
import concourse.mybir as mybir
import concourse.tile as tile
from concourse import bacc, bass_utils

dt = mybir.dt
F32, F16, F32R = dt.float32, dt.float16, dt.float32r
AF = mybir.ActivationFunctionType

DIM, HEADS, DHEAD, CTX_DIM, DFF = 1280, 20, 64, 768, 5120
BATCH, NTOK, MCTX = 4, 1024, 77
EPS = 1e-5
SCALE = DHEAD ** -0.5
N_CORES = 8
T = 512         # local tokens per core
TKV = 1024      # attn1 key/value tokens per core
KC = DIM // 128           # 10
KCX = CTX_DIM // 128      # 6
JFF = DFF // 128          # 40 (chunks of the gated hidden)
P = 128

last_exec_time_ns = None


def _emit(tc, d, trivial_aff, trivial_bias):
    nc = tc.nc
    pools = {}

    def pool(name, bufs, space="SBUF", side="left"):
        p = tc.alloc_tile_pool(name=name, bufs=bufs, space=space, side=side)
        pools[name] = p
        return p

    def close(*names):
        for n in names:
            pools.pop(n).release()

    # Pools are two LIFO stacks (left/right) per memory space; lifetimes below
    # are arranged so every release pops the top of its stack.
    const = pool("const", 1)
    ones_col = const.tile([P, 1], F16, name="ones_col")
    nc.vector.memset(ones_col[:], 1.0)
    ones_row = const.tile([1, P], F16, name="ones_row")
    nc.vector.memset(ones_row[:], 1.0)
    if not trivial_aff:
        aff = const.tile([P, 60], F32, name="aff")
        nc.sync.dma_start(aff[:], d["aff"])
    if not trivial_bias:
        biases = const.tile([P, 110], F32, name="biases")
        nc.sync.dma_start(biases[:], d["biases"])

    tmp = pool("tmp", 1)

    # ---------------- helpers ----------------

    def layernorm(x_tiles, Ttok, ln_idx, out_tiles, ln_psum):
        """x_tiles: KC SBUF tiles [128, Ttok] f32 -> out_tiles [128, Ttok] fp16."""
        for t in range(Ttok // 512):
            sl = slice(t * 512, (t + 1) * 512)
            sums_ps = ln_psum.tile([1, 512], F32, name=f"lns{ln_idx}_{t}", tag="lnstat", bufs=2)
            sq_ps = ln_psum.tile([1, 512], F32, name=f"lnq{ln_idx}_{t}", tag="lnstat", bufs=2)
            xhs = []
            for c in range(KC):
                xh = tmp.tile([P, 512], F16, name=f"xh{ln_idx}_{t}_{c}", tag="xh", bufs=10)
                nc.scalar.copy(xh[:], x_tiles[c][:, sl])
                xhs.append(xh)
                xsq = tmp.tile([P, 512], F16, name=f"xsq{ln_idx}_{t}_{c}", tag="xsq", bufs=3)
                nc.gpsimd.tensor_mul(xsq[:], xh[:], xh[:])
                nc.tensor.matmul(sums_ps[:], ones_col[:], xh[:],
                                 start=(c == 0), stop=(c == KC - 1))
                nc.tensor.matmul(sq_ps[:], ones_col[:], xsq[:],
                                 start=(c == 0), stop=(c == KC - 1))
            ssum = tmp.tile([1, 512], F16, name=f"ssum{ln_idx}_{t}", tag="ssum", bufs=2)
            nc.scalar.copy(ssum[:], sums_ps[:])
            ssq = tmp.tile([1, 512], F16, name=f"ssq{ln_idx}_{t}", tag="ssq", bufs=2)
            nc.scalar.copy(ssq[:], sq_ps[:])
            bs_ps = ln_psum.tile([P, 512], F32, name=f"bs{ln_idx}_{t}", tag="lnbc", bufs=2)
            nc.tensor.matmul(bs_ps[:], ones_row[:], ssum[:], start=True, stop=True)
            bq_ps = ln_psum.tile([P, 512], F32, name=f"bq{ln_idx}_{t}", tag="lnbc", bufs=2)
            nc.tensor.matmul(bq_ps[:], ones_row[:], ssq[:], start=True, stop=True)
            mu = tmp.tile([P, 512], F32, name=f"mu{ln_idx}_{t}", tag="mu", bufs=2)
            nc.vector.tensor_scalar_mul(mu[:], bs_ps[:], 1.0 / DIM)
            musq = tmp.tile([P, 512], F32, name=f"musq{ln_idx}_{t}", tag="musq", bufs=1)
            nc.vector.tensor_mul(musq[:], mu[:], mu[:])
            # musq - EPS, so var = ex2 - musq + EPS below
            nc.vector.tensor_scalar_sub(musq[:], musq[:], EPS)
            var = tmp.tile([P, 512], F32, name=f"var{ln_idx}_{t}", tag="var", bufs=1)
            nc.vector.scalar_tensor_tensor(var[:], bq_ps[:], 1.0 / DIM, musq[:],
                                           mybir.AluOpType.mult, mybir.AluOpType.subtract)
            std = tmp.tile([P, 512], F32, name=f"std{ln_idx}_{t}", tag="std", bufs=1)
            nc.scalar.sqrt(std[:], var[:])
            rstd = tmp.tile([P, 512], F32, name=f"rstd{ln_idx}_{t}", tag="rstd", bufs=2)
            nc.vector.reciprocal_approx_fast(rstd[:], std[:])
            rstd16 = tmp.tile([P, 512], F16, name=f"rstd16{ln_idx}_{t}", tag="rstd16", bufs=2)
            nc.vector.tensor_copy(out=rstd16[:], in_=rstd[:])
            mu16 = tmp.tile([P, 512], F16, name=f"mu16{ln_idx}_{t}", tag="mu16", bufs=2)
            nc.vector.tensor_copy(out=mu16[:], in_=mu[:])
            for c in range(KC):
                xm = tmp.tile([P, 512], F16, name=f"xm{ln_idx}_{t}_{c}", tag="xm", bufs=3)
                nc.vector.tensor_sub(xm[:], xhs[c][:], mu16[:])
                if trivial_aff:
                    nc.vector.tensor_mul(out_tiles[c][:, sl], xm[:], rstd16[:])
                else:
                    xn = tmp.tile([P, 512], F16, name=f"xn{ln_idx}_{t}_{c}", tag="xn", bufs=3)
                    nc.vector.tensor_mul(xn[:], xm[:], rstd16[:])
                    g_ap = aff[:, ln_idx * 20 + c: ln_idx * 20 + c + 1]
                    be_ap = aff[:, ln_idx * 20 + 10 + c: ln_idx * 20 + 10 + c + 1]
                    xg = tmp.tile([P, 512], F16, name=f"xg{ln_idx}_{t}_{c}", tag="xg", bufs=3)
                    nc.vector.tensor_scalar_mul(xg[:], xn[:], g_ap)
                    nc.scalar.activation(out_tiles[c][:, sl], xg[:], AF.Copy, bias=be_ap)

    def attn_finish(head, ops_, ov_psum, out_ap, evac_act=False):
        usb = tmp.tile([DHEAD + 1, 512], F16, name=f"usb{head}", tag="usb", bufs=4)
        if evac_act:
            nc.scalar.copy(usb[:], ops_[:])
        else:
            nc.vector.tensor_copy(out=usb[:], in_=ops_[:])
        den = tmp.tile([1, 512], F32, name=f"den{head}", tag="den", bufs=3)
        if evac_act:
            nc.scalar.copy(den[:], usb[DHEAD:DHEAD + 1, :])
        else:
            nc.vector.tensor_copy(out=den[:], in_=usb[DHEAD:DHEAD + 1, :])
        rec32 = tmp.tile([1, 512], F32, name=f"rec32_{head}", tag="rec32", bufs=3)
        nc.vector.reciprocal_approx_fast(rec32[:], den[:])
        rec = tmp.tile([1, 512], F16, name=f"rec{head}", tag="rec", bufs=3)
        if evac_act:
            nc.scalar.copy(rec[:], rec32[:])
        else:
            nc.vector.tensor_copy(out=rec[:], in_=rec32[:])
        bps = ov_psum.tile([DHEAD, 512], F32, name=f"bps{head}", tag="ov", bufs=2)
        nc.tensor.matmul(bps[:], ones_row[:, :DHEAD], rec[:],
                         start=True, stop=True)
        nc.vector.tensor_mul(out_ap, usb[:DHEAD, :], bps[:])

    def attn_pipeline(Kt_, Qt_, Vt_, n_kc, kv_par, sc_psum, ov_psum, epool, O_out,
                      fillers=()):
        """Both heads of pair c share one two-bank score PSUM tile per key
        chunk (one exp instruction covers both heads). attnV of an earlier
        pair is interleaved with pair c's score matmuls at key-chunk
        granularity so the PE streams at the ACT exp pace, and `fillers`
        emit independent PE work (the V projection) after the first pairs'
        scores to keep the PE warm while exps accumulate."""
        depth = 2 if n_kc > 1 else 1
        evac_act = n_kc == 1
        pend = []  # (pair_idx, exps) awaiting attnV

        def alloc_ov(pc):
            return [ov_psum.tile([DHEAD + 1, 512], F32, name=f"ov{2 * pc + h}",
                                 tag="ov", bufs=2) for h in range(2)]

        def av_mm(pc, pexps, ov, k8):
            for h in range(2):
                nc.tensor.matmul(ov[h][:], Vt_[k8][:kv_par, 2 * pc + h, :],
                                 pexps[k8][:, h * 512:(h + 1) * 512],
                                 start=(k8 == 0), stop=(k8 == n_kc - 1))

        def finish_pair(pc, ov):
            attn_finish(2 * pc, ov[0], ov_psum, O_out[pc][0:DHEAD, :], evac_act)
            attn_finish(2 * pc + 1, ov[1], ov_psum, O_out[pc][DHEAD:2 * DHEAD, :],
                        evac_act)

        for c in range(KC):
            drain = pend.pop(0) if len(pend) >= depth else None
            dov = alloc_ov(drain[0]) if drain else None
            exps = []
            for k8 in range(n_kc):
                sps = sc_psum.tile([kv_par, 1024], F32, name=f"sps{c}_{k8}", tag="sc",
                                   bufs=2)
                for h in range(2):
                    nc.tensor.matmul(sps[:, h * 512:(h + 1) * 512],
                                     Kt_[c][64 * h:64 * h + 64,
                                            k8 * kv_par:(k8 + 1) * kv_par],
                                     Qt_[c][64 * h:64 * h + 64, :],
                                     start=True, stop=True, tile_position=(64 * h, 0))
                e = epool.tile([kv_par, 1024], F16, name=f"exp{c}_{k8}", tag="exp")
                nc.scalar.activation(e[:], sps[:], AF.Exp, scale=SCALE)
                exps.append(e)
                if drain is not None:
                    av_mm(drain[0], drain[1], dov, k8)
            if drain is not None:
                finish_pair(drain[0], dov)
            if c < len(fillers):
                fillers[c]()
            pend.append((c, exps))
        for pc, pexps in pend:
            ov = alloc_ov(pc)
            for k8 in range(n_kc):
                av_mm(pc, pexps, ov, k8)
            finish_pair(pc, ov)

    def project(w_d, n_kc, rhs_fn, n_mc, consume, wpool, wtag, psum_p, wbufs=3):
        """out[mc] = sum_kc w[mc][:, kc].T @ rhs(kc); consume(mc, psum)."""
        for mc in range(n_mc):
            wt = wpool.tile([P, n_kc, P], F16, name=f"{wtag}_{mc}", tag=wtag, bufs=wbufs)
            nc.sync.dma_start(wt[:], w_d[mc])
            ps = psum_p.tile([P, 512], F32, name=f"ps_{wtag}_{mc}", tag="proj", bufs=4)
            for kc in range(n_kc):
                nc.tensor.matmul(ps[:], wt[:, kc], rhs_fn(kc),
                                 start=(kc == 0), stop=(kc == n_kc - 1))
            consume(mc, ps)

    def bias_ap(col):
        return biases[:, col:col + 1]

    # ---------------- phase 1: load x, LN1 ----------------

    otp = pool("otp", 1)
    ln1p = pool("ln1p", 1)
    ln1t = [ln1p.tile([P, TKV], F16, name=f"ln1_{c}", tag="ln1", bufs=KC) for c in range(KC)]

    ln_psum = pool("ln_psum", 1, space="PSUM")
    xpool = pool("xpool", 1, side="right")
    x_sb = []
    for c in range(KC):
        xc = xpool.tile([P, TKV], F32, name=f"x_{c}", tag="x", bufs=KC)
        nc.sync.dma_start(xc[:], d["xt"][c * P:(c + 1) * P, :])
        x_sb.append(xc)
    layernorm(x_sb, TKV, 0, ln1t, ln_psum)
    close("xpool", "ln_psum")

    # ---------------- phase 2: Q, K projections ----------------

    proj_psum = pool("proj_psum", 1, space="PSUM")
    wpool = pool("wpool1", 1)
    qkv = pool("qkv", 1, side="right")

    Qt = [qkv.tile([P, T], F16, name=f"qt_{mc}", tag="qt", bufs=KC) for mc in range(KC)]
    Kt = [qkv.tile([P, TKV], F16, name=f"kt_{mc}", tag="kt", bufs=KC) for mc in range(KC)]
    Vt = [qkv.tile([P, HEADS, DHEAD + 1], F16, name=f"vt_{t8}", tag="vt", bufs=8)
          for t8 in range(8)]
    Ot = [otp.tile([P, T], F16, name=f"ot_{c}", tag="ot", bufs=KC) for c in range(KC)]

    def q_consume(mc, ps):
        nc.vector.tensor_copy(out=Qt[mc][:], in_=ps[:])

    project(d["wq1"], KC, lambda kc: ln1t[kc][:, 0:T], KC, q_consume, wpool, "wq1", proj_psum)

    for thalf in range(2):
        sl = slice(thalf * 512, (thalf + 1) * 512)

        def k_consume(mc, ps, sl=sl):
            nc.vector.tensor_copy(out=Kt[mc][:, sl], in_=ps[:])

        project(d["wk1"], KC, lambda kc, sl=sl: ln1t[kc][:, sl], KC, k_consume, wpool, "wk1",
                proj_psum)

    close("proj_psum")

    # ---------------- phase 3: attn1 (V~ projection runs as filler) ----------------

    sc_psum = pool("sc_psum", 1, space="PSUM")
    ov_psum = pool("ov_psum", 1, space="PSUM")
    vp_psum = pool("vp_psum", 1, space="PSUM")
    epool = pool("epool", 20, side="right")

    def vproj_filler(nt):
        n0, nsz = ((0, 512), (512, 512), (1024, 256))[nt]

        def run():
            if nt == 0:
                for t8 in range(8):
                    nc.vector.memset(Vt[t8][:], 1.0)
            wv_sl = []
            for kc in range(KC):
                wv = wpool.tile([P, 512], F16, name=f"wv1_{nt}_{kc}", tag="wv1", bufs=KC)
                nc.sync.dma_start(wv[:, :nsz], d["wv1"][kc][:, n0:n0 + nsz])
                wv_sl.append(wv)
            for t8 in range(8):
                ps = vp_psum.tile([P, 512], F32, name=f"psv_{t8}_{n0}", tag="vproj",
                                  bufs=2)
                for kc in range(KC):
                    nc.tensor.matmul(ps[:, :nsz], ln1t[kc][:, t8 * P:(t8 + 1) * P],
                                     wv_sl[kc][:, :nsz],
                                     start=(kc == 0), stop=(kc == KC - 1))
                nc.vector.tensor_copy(
                    out=Vt[t8][:, n0 // DHEAD:(n0 + nsz) // DHEAD, 0:DHEAD],
                    in_=ps[:, :nsz].rearrange("p (h e) -> p h e", e=DHEAD))
        return run

    attn_pipeline(Kt, Qt, Vt, 8, P, sc_psum, ov_psum, epool, Ot,
                  fillers=(vproj_filler(0), vproj_filler(1), vproj_filler(2)))

    close("epool", "qkv", "vp_psum", "ov_psum", "sc_psum", "wpool1", "ln1p")

    # ---------------- phase 4: out-proj 1 + residual ----------------

    resp = pool("resp", 1)
    wpool = pool("wpool2", 1)
    proj_psum = pool("proj_psum2", 1, space="PSUM")
    x1p = pool("x1p", 1, side="right")
    x1 = [x1p.tile([P, T], F32, name=f"x1_{mc}", tag="x1", bufs=KC) for mc in range(KC)]
    resid = []
    for c in range(KC):
        rc = resp.tile([P, T], F32, name=f"res_{c}", tag="res", bufs=KC)
        nc.sync.dma_start(rc[:], d["xres"][c * P:(c + 1) * P, :])
        resid.append(rc)

    def o1_consume(mc, ps):
        if trivial_bias:
            nc.vector.tensor_add(x1[mc][:], ps[:], resid[mc][:])
        else:
            nc.vector.scalar_tensor_tensor(x1[mc][:], ps[:], bias_ap(mc), resid[mc][:],
                                           mybir.AluOpType.add, mybir.AluOpType.add)

    project(d["wo1"], KC, lambda kc: Ot[kc][:], KC, o1_consume, wpool, "wo1", proj_psum)
    close("wpool2", "resp", "otp", "proj_psum2")

    # ---------------- phase 5: LN2 + attn2 projections ----------------

    o2p = pool("o2p", 1)
    wpool = pool("wpool2b", 1)
    ln_psum = pool("ln_psum2", 1, space="PSUM")
    ln2p = pool("ln2p", 1)
    ln2t = [ln2p.tile([P, T], F16, name=f"ln2_{c}", tag="ln2", bufs=KC) for c in range(KC)]
    layernorm(x1, T, 1, ln2t, ln_psum)
    close("ln_psum2")

    proj_psum = pool("proj_psum2b", 1, space="PSUM")
    qkv2 = pool("qkv2", 1, side="right")
    ctx_sb = []
    for c in range(KCX):
        cc = qkv2.tile([P, MCTX], F32, name=f"ctx_{c}", tag="ctx", bufs=KCX)
        nc.sync.dma_start(cc[:], d["ctxt"][c * P:(c + 1) * P, :])
        ch = qkv2.tile([P, MCTX], F16, name=f"ctxh_{c}", tag="ctxh", bufs=KCX)
        nc.any.tensor_copy(out=ch[:], in_=cc[:])
        ctx_sb.append(ch)

    Q2t = [qkv2.tile([P, T], F16, name=f"q2t_{mc}", tag="q2t", bufs=KC) for mc in range(KC)]
    K2t = [qkv2.tile([P, MCTX], F16, name=f"k2t_{mc}", tag="k2t", bufs=KC) for mc in range(KC)]
    V2t = [qkv2.tile([P, HEADS, DHEAD + 1], F16, name="v2t", tag="v2t", bufs=1)]
    O2t = [o2p.tile([P, T], F16, name=f"o2t_{c}", tag="o2t", bufs=KC) for c in range(KC)]

    def q2_consume(mc, ps):
        nc.any.tensor_copy(out=Q2t[mc][:], in_=ps[:])

    project(d["wq2"], KC, lambda kc: ln2t[kc][:], KC, q2_consume, wpool, "wq2", proj_psum)

    for mc in range(KC):
        wt = wpool.tile([P, KCX, P], F16, name=f"wk2_{mc}", tag="wk2", bufs=3)
        nc.sync.dma_start(wt[:], d["wk2"][mc])
        ps = proj_psum.tile([P, MCTX], F32, name=f"psk2_{mc}", tag="projx", bufs=2)
        for kc in range(KCX):
            nc.tensor.matmul(ps[:], wt[:, kc], ctx_sb[kc][:], start=(kc == 0),
                             stop=(kc == KCX - 1))
        nc.any.tensor_copy(out=K2t[mc][:], in_=ps[:])

    wv2_sb = []
    for kc in range(KCX):
        wv = wpool.tile([P, DIM], F16, name=f"wv2_{kc}", tag="wv2", bufs=KCX)
        nc.sync.dma_start(wv[:], d["wv2"][kc])
        wv2_sb.append(wv)

    close("ln2p", "proj_psum2b")

    # ---------------- phase 6: attn2 ----------------

    sc_psum = pool("sc_psum2", 1, space="PSUM")
    ov_psum = pool("ov_psum2", 1, space="PSUM")
    vp_psum = pool("vp_psum2", 1, space="PSUM")
    epool = pool("epool2", 6, side="right")

    def v2proj_filler():
        nc.vector.memset(V2t[0][:], 1.0)
        for n0, nsz in ((0, 512), (512, 512), (1024, 256)):
            ps = vp_psum.tile([MCTX, 512], F32, name=f"psv2_{n0}", tag="vproj", bufs=2)
            for kc in range(KCX):
                nc.tensor.matmul(ps[:, :nsz], ctx_sb[kc][:], wv2_sb[kc][:, n0:n0 + nsz],
                                 start=(kc == 0), stop=(kc == KCX - 1))
            nc.any.tensor_copy(
                out=V2t[0][:MCTX, n0 // DHEAD:(n0 + nsz) // DHEAD, 0:DHEAD],
                in_=ps[:, :nsz].rearrange("p (h e) -> p h e", e=DHEAD))

    attn_pipeline(K2t, Q2t, V2t, 1, MCTX, sc_psum, ov_psum, epool, O2t,
                  fillers=(v2proj_filler,))

    close("epool2", "qkv2", "vp_psum2", "ov_psum2", "sc_psum2", "wpool2b")

    # ---------------- phase 7: out-proj 2 + residual ----------------

    x2p = pool("x2p", 1)
    wpool = pool("wpool3", 1)
    proj_psum = pool("proj_psum3", 1, space="PSUM")
    x2 = [x2p.tile([P, T], F32, name=f"x2_{mc}", tag="x2", bufs=KC) for mc in range(KC)]

    def o2_consume(mc, ps):
        if trivial_bias:
            nc.vector.tensor_add(x2[mc][:], ps[:], x1[mc][:])
        else:
            nc.vector.scalar_tensor_tensor(x2[mc][:], ps[:], bias_ap(10 + mc), x1[mc][:],
                                           mybir.AluOpType.add, mybir.AluOpType.add)

    project(d["wo2"], KC, lambda kc: O2t[kc][:], KC, o2_consume, wpool, "wo2", proj_psum)
    close("wpool3", "x1p", "proj_psum3")

    # ---------------- phase 8: LN3 + GEGLU FF ----------------

    hhp = pool("hhp", 1)
    hht = [hhp.tile([P, T], F16, name=f"hh_{j}", tag="hh", bufs=JFF) for j in range(JFF)]

    ln_psum = pool("ln_psum3", 1, space="PSUM")
    ln3p = pool("ln3p", 1)
    ln3t = [ln3p.tile([P, T], F16, name=f"ln3_{c}", tag="ln3", bufs=KC) for c in range(KC)]
    layernorm(x2, T, 2, ln3t, ln_psum)
    close("ln_psum3")

    wpool = pool("wpool4a", 1)
    proj_psum = pool("proj_psum4", 1, space="PSUM")
    for j in range(JFF):
        wg = wpool.tile([P, KC, P], F16, name=f"wg_{j}", tag="wff1g", bufs=3)
        nc.sync.dma_start(wg[:], d["wff1"][JFF + j])
        gps = proj_psum.tile([P, 512], F32, name=f"gps_{j}", tag="proj", bufs=4)
        for kc in range(KC):
            nc.tensor.matmul(gps[:], wg[:, kc], ln3t[kc][:], start=(kc == 0),
                             stop=(kc == KC - 1))
        gel = tmp.tile([P, T], F16, name=f"gel_{j}", tag="gel", bufs=3)
        if trivial_bias:
            nc.scalar.activation(gel[:], gps[:], AF.Gelu_apprx_tanh)
        else:
            nc.scalar.activation(gel[:], gps[:], AF.Gelu_apprx_tanh, bias=bias_ap(60 + j))

        wa = wpool.tile([P, KC, P], F16, name=f"wa_{j}", tag="wff1a", bufs=3)
        nc.sync.dma_start(wa[:], d["wff1"][j])
        aps = proj_psum.tile([P, 512], F32, name=f"aps_{j}", tag="proj", bufs=4)
        for kc in range(KC):
            nc.tensor.matmul(aps[:], wa[:, kc], ln3t[kc][:], start=(kc == 0),
                             stop=(kc == KC - 1))
        if trivial_bias:
            nc.vector.tensor_mul(hht[j][:], aps[:], gel[:])
        else:
            nc.vector.scalar_tensor_tensor(hht[j][:], aps[:], bias_ap(20 + j), gel[:],
                                           mybir.AluOpType.add, mybir.AluOpType.mult)

    close("wpool4a", "ln3p")

    # ---------------- phase 9: FF down-proj + residual -> out ----------------

    wpool = pool("wpool4b", 1)
    outp = pool("outp", 4)
    for mc in range(KC):
        wt = wpool.tile([P, JFF, P], F16, name=f"wff2_{mc}", tag="wff2", bufs=2)
        nc.sync.dma_start(wt[:], d["wff2"][mc])
        ps = proj_psum.tile([P, 512], F32, name=f"psf2_{mc}", tag="proj", bufs=4)
        for kc in range(JFF):
            nc.tensor.matmul(ps[:], wt[:, kc], hht[kc][:], start=(kc == 0),
                             stop=(kc == JFF - 1))
        ot = outp.tile([P, T], F32, name=f"out_{mc}", tag="out")
        if trivial_bias:
            nc.vector.tensor_add(ot[:], ps[:], x2[mc][:])
        else:
            nc.vector.scalar_tensor_tensor(ot[:], ps[:], bias_ap(100 + mc), x2[mc][:],
                                           mybir.AluOpType.add, mybir.AluOpType.add)
        nc.sync.dma_start(d["out"][mc * P:(mc + 1) * P, :], ot[:])

    close("outp", "wpool4b", "hhp", "x2p", "o2p", "tmp", "const", "proj_psum4")


def _lhst_layout(w, n_kc, n_mc):
    """[K, M] f32 -> fp16 [n_mc, 128, n_kc, 128] so block [mc] is the
    contiguous stationary-operand group for output chunk mc."""
    return np.ascontiguousarray(
        w.reshape(n_kc, P, n_mc, P).transpose(2, 1, 0, 3).astype(np.float16))


def _rhs_layout(w, n_kc):
    """[K, M] f32 -> fp16 [n_kc, 128, M] row-chunk (moving-operand) layout."""
    return np.ascontiguousarray(w.reshape(n_kc, P, -1).astype(np.float16))


_BUILT = {}


def _build(trivial_aff, trivial_bias):
    key = (trivial_aff, trivial_bias)
    if key in _BUILT:
        return _BUILT[key]
    nc = bacc.Bacc("TRN2", target_bir_lowering=False, debug=False, num_devices=N_CORES)
    d = {
        "xt": nc.dram_tensor("xt", [DIM, TKV], F32, kind="ExternalInput").ap(),
        "ctxt": nc.dram_tensor("ctxt", [CTX_DIM, MCTX], F32, kind="ExternalInput").ap(),
        "xres": nc.dram_tensor("xres", [DIM, T], F32, kind="ExternalInput").ap(),
        "wq1": nc.dram_tensor("wq1", [KC, P, KC, P], F16, kind="ExternalInput").ap(),
        "wk1": nc.dram_tensor("wk1", [KC, P, KC, P], F16, kind="ExternalInput").ap(),
        "wv1": nc.dram_tensor("wv1", [KC, P, DIM], F16, kind="ExternalInput").ap(),
        "wo1": nc.dram_tensor("wo1", [KC, P, KC, P], F16, kind="ExternalInput").ap(),
        "wq2": nc.dram_tensor("wq2", [KC, P, KC, P], F16, kind="ExternalInput").ap(),
        "wk2": nc.dram_tensor("wk2", [KC, P, KCX, P], F16, kind="ExternalInput").ap(),
        "wv2": nc.dram_tensor("wv2", [KCX, P, DIM], F16, kind="ExternalInput").ap(),
        "wo2": nc.dram_tensor("wo2", [KC, P, KC, P], F16, kind="ExternalInput").ap(),
        "wff1": nc.dram_tensor("wff1", [2 * JFF, P, KC, P], F16, kind="ExternalInput").ap(),
        "wff2": nc.dram_tensor("wff2", [KC, P, JFF, P], F16, kind="ExternalInput").ap(),
        "out": nc.dram_tensor("out", [DIM, T], F32, kind="ExternalOutput").ap(),
    }
    if not trivial_aff:
        d["aff"] = nc.dram_tensor("aff", [P, 60], F32, kind="ExternalInput").ap()
    if not trivial_bias:
        d["biases"] = nc.dram_tensor("biases", [P, 110], F32, kind="ExternalInput").ap()
    with tile.TileContext(nc) as tc:
        _emit(tc, d, trivial_aff, trivial_bias)
    nc.compile()
    _BUILT[key] = nc
    return nc


def kernel(x, context,
           g1, be1, wq1, wk1, wv1, wo1, bo1,
           g2, be2, wq2, wk2, wv2, wo2, bo2,
           g3, be3, w_ff1, b_ff1, w_ff2, b_ff2,
           _trace=False):
    global last_exec_time_ns
    x = np.asarray(x, np.float32)
    context = np.asarray(context, np.float32)

    affs = [np.asarray(a, np.float32) for a in (g1, be1, g2, be2, g3, be3)]
    biases = [np.asarray(b, np.float32) for b in (bo1, bo2, b_ff1, b_ff2)]
    trivial_aff = all(np.all(a == (1.0 if i % 2 == 0 else 0.0))
                      for i, a in enumerate(affs))
    trivial_bias = all(np.all(b == 0.0) for b in biases)

    nc = _build(trivial_aff, trivial_bias)

    shared = {
        "wq1": _lhst_layout(np.asarray(wq1, np.float32), KC, KC),
        "wk1": _lhst_layout(np.asarray(wk1, np.float32), KC, KC),
        "wv1": _rhs_layout(np.asarray(wv1, np.float32), KC),
        "wo1": _lhst_layout(np.asarray(wo1, np.float32), KC, KC),
        "wq2": _lhst_layout(np.asarray(wq2, np.float32), KC, KC),
        "wk2": _lhst_layout(np.asarray(wk2, np.float32), KCX, KC),
        "wv2": _rhs_layout(np.asarray(wv2, np.float32), KCX),
        "wo2": _lhst_layout(np.asarray(wo2, np.float32), KC, KC),
        "wff1": _lhst_layout(np.asarray(w_ff1, np.float32), KC, 2 * JFF),
        "wff2": _lhst_layout(np.asarray(w_ff2, np.float32), JFF, KC),
    }
    if not trivial_aff:
        aff = np.zeros([P, 60], np.float32)
        for i, a in enumerate(affs):
            # col = ln_idx*20 + (0 for g / 10 for be) + chunk
            ln_idx, j = i // 2, i % 2
            aff[:, ln_idx * 20 + j * 10: ln_idx * 20 + j * 10 + 10] = \
                a.reshape(KC, P).T
        shared["aff"] = aff
    if not trivial_bias:
        bb = np.zeros([P, 110], np.float32)
        bb[:, 0:10] = biases[0].reshape(KC, P).T
        bb[:, 10:20] = biases[1].reshape(KC, P).T
        bb[:, 20:100] = biases[2].reshape(2 * JFF, P).T
        bb[:, 100:110] = biases[3].reshape(KC, P).T
        shared["biases"] = bb

    in_maps = []
    for b in range(BATCH):
        ctxt = np.ascontiguousarray(context[b].T)
        for h in range(2):
            xr = np.roll(x[b], -h * T, axis=0)
            m = dict(shared)
            xrt = np.ascontiguousarray(xr.T)
            m["xt"] = xrt
            m["xres"] = np.ascontiguousarray(xrt[:, 0:T])
            m["ctxt"] = ctxt
            in_maps.append(m)

    res = bass_utils.run_bass_kernel_spmd(
        nc, in_maps, core_ids=list(range(N_CORES)), trace=_trace)
    last_exec_time_ns = res.exec_time_ns

    out = np.empty((BATCH, NTOK, DIM), np.float32)
    for b in range(BATCH):
        for h in range(2):
            out[b, h * T:(h + 1) * T, :] = res.results[b * 2 + h]["out"].T
    return out



# revision 3
# speedup vs baseline: 1.1092x; 1.1092x over previous
"""BasicTransformerBlock on 8 TRN2 NeuronCores.

Sharding: data-parallel, core = (batch b in 0..3) x (sequence half h in 0..1).
Each core receives its batch element's full sequence rotated so its local 512
rows come first (softmax over keys is permutation invariant), computes K/V of
attn1 for all 1024 tokens (duplicated across the pair, ~10% extra FLOPs, zero
collectives), and everything else for its 512 local tokens only.

On-chip layout: feature-major activations [features on partitions, tokens on
free axis] so every projection consumes natural-layout weights as the matmul
stationary operand.

Precision: the attention path (Q/K/V/O projections, attnV) runs in fp8 e4m3
with DoubleRow dual-fp8 matmuls (2x PE throughput). Weights are pre-scaled by
16 on the host so they sit in e4m3's normal range; Q/K/V keep the x16 scale in
their fp8 tiles (scores come out x256, folded into the exp scale; the V ones-
column is 16 so the denominator cancels it). The GEGLU FF stays fp16 (fp8
there costs ~1.5e-2 relative error, over budget). The residual stream, LN
math and PSUM accumulation stay fp32. LayerNorm partition reductions use
dual-fp8 ones-matmuls; per-token broadcasts use fp16 ones-matmuls; softmax
denominators come free from a ones-column appended to V.

Dual-fp8 DoubleRow matmuls pair two 128-deep contractions per instruction;
hardware requires the pair stride in both operands to be a multiple of 16
bytes and PSUM outputs to start at partition 0 (hence the 1312-wide padded V
tile and the 80-wide padded context tile).
"""

import sys
import types

sys.path.insert(0, "/opt/trn_rl_repo")

# concourse fetches the NTFF profile hook from antenv.axon_hooks, which the
# agent image's antenv stub lacks. Register a shim so trace=True works.
if "antenv.axon_hooks" not in sys.modules:
    _hooks = types.ModuleType("antenv.axon_hooks")
    _HOOK = [None]

    def _get_hook():
        if _HOOK[0] is None:
            try:
                from trn_agent_boot.trn_boot import _ntff_profile_via_ctypes

                _HOOK[0] = _ntff_profile_via_ctypes("/opt/axon/libaxon_pjrt.so")
            except Exception:
                _HOOK[0] = None
        return _HOOK[0]

    _hooks.get_axon_ntff_profile_hook = _get_hook
    _hooks.set_axon_ntff_profile_hook = lambda h: _HOOK.__setitem__(0, h)
    sys.modules["antenv.axon_hooks"] = _hooks
    try:
        import antenv

        antenv.axon_hooks = _hooks
    except ImportError:
        pass

import ml_dtypes
import numpy as np

import concourse.bass as bass
import concourse.mybir as mybir
import concourse.tile as tile
from concourse import bacc, bass_utils

dt = mybir.dt
F32, F16, F8 = dt.float32, dt.float16, dt.float8e4
AF = mybir.ActivationFunctionType
DR = mybir.MatmulPerfMode.DoubleRow
E4 = ml_dtypes.float8_e4m3

DIM, HEADS, DHEAD, CTX_DIM, DFF = 1280, 20, 64, 768, 5120
BATCH, NTOK, MCTX = 4, 1024, 77
EPS = 1e-5
SCALE = DHEAD ** -0.5
N_CORES = 8
T = 512         # local tokens per core
TKV = 1024      # attn1 key/value tokens per core
KC = DIM // 128           # 10
KP = KC // 2              # 5 dual-fp8 k-chunk pairs
KCX = CTX_DIM // 128      # 6
KPX = KCX // 2            # 3
JFF = DFF // 128          # 40 (chunks of the gated hidden)
P = 128
WS = 16.0                 # host-side fp8 weight scale
EBIAS = -1.0              # exp bias (cancels in softmax, keeps exps in range)
VSTRIDE = 1312            # padded 20*(DHEAD+1)=1300 -> 16B-aligned pair stride

last_exec_time_ns = None


def _emit(tc, d, trivial_aff, trivial_bias):
    nc = tc.nc
    pools = {}

    def pool(name, bufs, space="SBUF", side="left"):
        p = tc.alloc_tile_pool(name=name, bufs=bufs, space=space, side=side)
        pools[name] = p
        return p

    def close(*names):
        for n in names:
            pools.pop(n).release()

    # Pools are two LIFO stacks (left/right) per memory space; lifetimes below
    # are arranged so every release pops the top of its stack.
    const = pool("const", 1)
    ones_col = const.tile([P, 1], F16, name="ones_col")
    nc.vector.memset(ones_col[:], 1.0)
    ones_row = const.tile([1, P], F16, name="ones_row")
    nc.vector.memset(ones_row[:], 1.0)
    # dual-fp8 ones pair for LN stat reductions (16B-aligned pair stride)
    ones8 = const.tile([P, 2, 16], F8, name="ones8")
    nc.vector.memset(ones8[:], 1.0)
    ebias = const.tile([P, 1], F32, name="ebias")
    nc.vector.memset(ebias[:], EBIAS)
    if not trivial_aff:
        aff = const.tile([P, 60], F32, name="aff")
        nc.sync.dma_start(aff[:], d["aff"])
    if not trivial_bias:
        biases = const.tile([P, 110], F32, name="biases")
        nc.sync.dma_start(biases[:], d["biases"])

    tmp = pool("tmp", 1)

    # ---------------- helpers ----------------

    def layernorm8(x_tiles, Ttok, ln_idx, out_big, ln_psum):
        """x_tiles: KC f32 SBUF tiles [128, Ttok] -> out_big fp8 [128, KC, Ttok].

        Stats run on fp8 copies via dual-fp8 ones-matmuls (mean/var noise is
        O(2.7%/sqrt(DIM)), negligible); the normalized output is fp8 anyway.
        """
        for t in range(Ttok // 512):
            sl = slice(t * 512, (t + 1) * 512)
            sums_ps = ln_psum.tile([1, 512], F32, name=f"lns{ln_idx}_{t}", tag="lnstat", bufs=2)
            sq_ps = ln_psum.tile([1, 512], F32, name=f"lnq{ln_idx}_{t}", tag="lnstat", bufs=2)
            xh8 = tmp.tile([P, KC, 512], F8, name=f"xh8_{ln_idx}_{t}", tag="xh8", bufs=2)
            for p8 in range(KP):
                xsq = tmp.tile([P, 2, 512], F8, name=f"xsq{ln_idx}_{t}_{p8}", tag="xsq", bufs=3)
                for s in range(2):
                    c = 2 * p8 + s
                    nc.scalar.copy(xh8[:, c, :], x_tiles[c][:, sl])
                    nc.vector.tensor_mul(xsq[:, s, :], xh8[:, c, :], xh8[:, c, :])
                nc.tensor.matmul(sums_ps[:], ones8[:, :, 0:1], xh8[:, 2 * p8:2 * p8 + 2, :],
                                 start=(p8 == 0), stop=(p8 == KP - 1), perf_mode=DR)
                nc.tensor.matmul(sq_ps[:], ones8[:, :, 0:1], xsq[:],
                                 start=(p8 == 0), stop=(p8 == KP - 1), perf_mode=DR)
            ssum = tmp.tile([1, 512], F16, name=f"ssum{ln_idx}_{t}", tag="ssum", bufs=2)
            nc.scalar.copy(ssum[:], sums_ps[:])
            ssq = tmp.tile([1, 512], F16, name=f"ssq{ln_idx}_{t}", tag="ssq", bufs=2)
            nc.scalar.copy(ssq[:], sq_ps[:])
            bs_ps = ln_psum.tile([P, 512], F32, name=f"bs{ln_idx}_{t}", tag="lnbc", bufs=2)
            nc.tensor.matmul(bs_ps[:], ones_row[:], ssum[:], start=True, stop=True)
            bq_ps = ln_psum.tile([P, 512], F32, name=f"bq{ln_idx}_{t}", tag="lnbc", bufs=2)
            nc.tensor.matmul(bq_ps[:], ones_row[:], ssq[:], start=True, stop=True)
            mu = tmp.tile([P, 512], F32, name=f"mu{ln_idx}_{t}", tag="mu", bufs=2)
            nc.vector.tensor_scalar_mul(mu[:], bs_ps[:], 1.0 / DIM)
            musq = tmp.tile([P, 512], F32, name=f"musq{ln_idx}_{t}", tag="musq", bufs=1)
            nc.vector.tensor_mul(musq[:], mu[:], mu[:])
            # musq - EPS, so var = ex2 - musq + EPS below
            nc.vector.tensor_scalar_sub(musq[:], musq[:], EPS)
            var = tmp.tile([P, 512], F32, name=f"var{ln_idx}_{t}", tag="var", bufs=1)
            nc.vector.scalar_tensor_tensor(var[:], bq_ps[:], 1.0 / DIM, musq[:],
                                           mybir.AluOpType.mult, mybir.AluOpType.subtract)
            std = tmp.tile([P, 512], F32, name=f"std{ln_idx}_{t}", tag="std", bufs=1)
            nc.scalar.sqrt(std[:], var[:])
            rstd = tmp.tile([P, 512], F32, name=f"rstd{ln_idx}_{t}", tag="rstd", bufs=2)
            nc.vector.reciprocal_approx_fast(rstd[:], std[:])
            rstd16 = tmp.tile([P, 512], F16, name=f"rstd16{ln_idx}_{t}", tag="rstd16", bufs=2)
            nc.vector.tensor_copy(out=rstd16[:], in_=rstd[:])
            mu16 = tmp.tile([P, 512], F16, name=f"mu16{ln_idx}_{t}", tag="mu16", bufs=2)
            nc.vector.tensor_copy(out=mu16[:], in_=mu[:])
            for c in range(KC):
                xm = tmp.tile([P, 512], F16, name=f"xm{ln_idx}_{t}_{c}", tag="xm", bufs=3)
                nc.vector.tensor_sub(xm[:], xh8[:, c, :], mu16[:])
                if trivial_aff:
                    nc.vector.tensor_mul(out_big[:, c, sl], xm[:], rstd16[:])
                else:
                    xn = tmp.tile([P, 512], F16, name=f"xn{ln_idx}_{t}_{c}", tag="xn", bufs=3)
                    nc.vector.tensor_mul(xn[:], xm[:], rstd16[:])
                    g_ap = aff[:, ln_idx * 20 + c: ln_idx * 20 + c + 1]
                    be_ap = aff[:, ln_idx * 20 + 10 + c: ln_idx * 20 + 10 + c + 1]
                    xg = tmp.tile([P, 512], F16, name=f"xg{ln_idx}_{t}_{c}", tag="xg", bufs=3)
                    nc.vector.tensor_scalar_mul(xg[:], xn[:], g_ap)
                    nc.scalar.activation(out_big[:, c, sl], xg[:], AF.Copy, bias=be_ap)

    def layernorm16(x_tiles, Ttok, ln_idx, out_tiles, ln_psum):
        """Baseline fp16 LN (used before the fp16 FF): fp16 stats + fp16 out."""
        for t in range(Ttok // 512):
            sl = slice(t * 512, (t + 1) * 512)
            sums_ps = ln_psum.tile([1, 512], F32, name=f"lns{ln_idx}_{t}", tag="lnstat", bufs=2)
            sq_ps = ln_psum.tile([1, 512], F32, name=f"lnq{ln_idx}_{t}", tag="lnstat", bufs=2)
            xhs = []
            for c in range(KC):
                xh = tmp.tile([P, 512], F16, name=f"xh{ln_idx}_{t}_{c}", tag="xh", bufs=10)
                nc.scalar.copy(xh[:], x_tiles[c][:, sl])
                xhs.append(xh)
                xsq = tmp.tile([P, 512], F16, name=f"xsqf{ln_idx}_{t}_{c}", tag="xsqf", bufs=3)
                nc.gpsimd.tensor_mul(xsq[:], xh[:], xh[:])
                nc.tensor.matmul(sums_ps[:], ones_col[:], xh[:],
                                 start=(c == 0), stop=(c == KC - 1))
                nc.tensor.matmul(sq_ps[:], ones_col[:], xsq[:],
                                 start=(c == 0), stop=(c == KC - 1))
            ssum = tmp.tile([1, 512], F16, name=f"ssum{ln_idx}_{t}", tag="ssum", bufs=2)
            nc.scalar.copy(ssum[:], sums_ps[:])
            ssq = tmp.tile([1, 512], F16, name=f"ssq{ln_idx}_{t}", tag="ssq", bufs=2)
            nc.scalar.copy(ssq[:], sq_ps[:])
            bs_ps = ln_psum.tile([P, 512], F32, name=f"bs{ln_idx}_{t}", tag="lnbc", bufs=2)
            nc.tensor.matmul(bs_ps[:], ones_row[:], ssum[:], start=True, stop=True)
            bq_ps = ln_psum.tile([P, 512], F32, name=f"bq{ln_idx}_{t}", tag="lnbc", bufs=2)
            nc.tensor.matmul(bq_ps[:], ones_row[:], ssq[:], start=True, stop=True)
            mu = tmp.tile([P, 512], F32, name=f"mu{ln_idx}_{t}", tag="mu", bufs=2)
            nc.vector.tensor_scalar_mul(mu[:], bs_ps[:], 1.0 / DIM)
            musq = tmp.tile([P, 512], F32, name=f"musq{ln_idx}_{t}", tag="musq", bufs=1)
            nc.vector.tensor_mul(musq[:], mu[:], mu[:])
            nc.vector.tensor_scalar_sub(musq[:], musq[:], EPS)
            var = tmp.tile([P, 512], F32, name=f"var{ln_idx}_{t}", tag="var", bufs=1)
            nc.vector.scalar_tensor_tensor(var[:], bq_ps[:], 1.0 / DIM, musq[:],
                                           mybir.AluOpType.mult, mybir.AluOpType.subtract)
            std = tmp.tile([P, 512], F32, name=f"std{ln_idx}_{t}", tag="std", bufs=1)
            nc.scalar.sqrt(std[:], var[:])
            rstd = tmp.tile([P, 512], F32, name=f"rstd{ln_idx}_{t}", tag="rstd", bufs=2)
            nc.vector.reciprocal_approx_fast(rstd[:], std[:])
            rstd16 = tmp.tile([P, 512], F16, name=f"rstd16{ln_idx}_{t}", tag="rstd16", bufs=2)
            nc.vector.tensor_copy(out=rstd16[:], in_=rstd[:])
            mu16 = tmp.tile([P, 512], F16, name=f"mu16{ln_idx}_{t}", tag="mu16", bufs=2)
            nc.vector.tensor_copy(out=mu16[:], in_=mu[:])
            for c in range(KC):
                xm = tmp.tile([P, 512], F16, name=f"xm{ln_idx}_{t}_{c}", tag="xm", bufs=3)
                nc.vector.tensor_sub(xm[:], xhs[c][:], mu16[:])
                if trivial_aff:
                    nc.vector.tensor_mul(out_tiles[c][:, sl], xm[:], rstd16[:])
                else:
                    xn = tmp.tile([P, 512], F16, name=f"xn{ln_idx}_{t}_{c}", tag="xn", bufs=3)
                    nc.vector.tensor_mul(xn[:], xm[:], rstd16[:])
                    g_ap = aff[:, ln_idx * 20 + c: ln_idx * 20 + c + 1]
                    be_ap = aff[:, ln_idx * 20 + 10 + c: ln_idx * 20 + 10 + c + 1]
                    xg = tmp.tile([P, 512], F16, name=f"xg{ln_idx}_{t}_{c}", tag="xg", bufs=3)
                    nc.vector.tensor_scalar_mul(xg[:], xn[:], g_ap)
                    nc.scalar.activation(out_tiles[c][:, sl], xg[:], AF.Copy, bias=be_ap)

    def attn_finish(head, ops_, ov_psum, out_ap, evac_act=False):
        usb = tmp.tile([DHEAD + 1, 512], F16, name=f"usb{head}", tag="usb", bufs=4)
        if evac_act:
            nc.scalar.copy(usb[:], ops_[:])
        else:
            nc.vector.tensor_copy(out=usb[:], in_=ops_[:])
        den = tmp.tile([1, 512], F32, name=f"den{head}", tag="den", bufs=3)
        if evac_act:
            nc.scalar.copy(den[:], usb[DHEAD:DHEAD + 1, :])
        else:
            nc.vector.tensor_copy(out=den[:], in_=usb[DHEAD:DHEAD + 1, :])
        rec32 = tmp.tile([1, 512], F32, name=f"rec32_{head}", tag="rec32", bufs=3)
        nc.vector.reciprocal_approx_fast(rec32[:], den[:])
        rec = tmp.tile([1, 512], F16, name=f"rec{head}", tag="rec", bufs=3)
        if evac_act:
            nc.scalar.copy(rec[:], rec32[:])
        else:
            nc.vector.tensor_copy(out=rec[:], in_=rec32[:])
        bps = ov_psum.tile([DHEAD, 512], F32, name=f"bps{head}", tag="ov", bufs=2)
        nc.tensor.matmul(bps[:], ones_row[:, :DHEAD], rec[:],
                         start=True, stop=True)
        nc.vector.tensor_mul(out_ap, usb[:DHEAD, :], bps[:])

    def project8(w_d, n_kp, rhs_fn, n_mc, consume, wpool, wtag, psum_p, wbufs=3,
                 nfree=512):
        """out[mc] = sum_p dualfp8( w[mc][:,p] , rhs(p) ); consume(mc, psum)."""
        for mc in range(n_mc):
            wt = wpool.tile([P, n_kp, 2, P], F8, name=f"{wtag}_{mc}", tag=wtag, bufs=wbufs)
            nc.sync.dma_start(wt[:], w_d[mc])
            ps = psum_p.tile([P, nfree], F32, name=f"ps_{wtag}_{mc}", tag="proj", bufs=4)
            for p8 in range(n_kp):
                nc.tensor.matmul(ps[:], wt[:, p8], rhs_fn(p8),
                                 start=(p8 == 0), stop=(p8 == n_kp - 1), perf_mode=DR)
            consume(mc, ps)

    def bias_ap(col):
        return biases[:, col:col + 1]

    # ---------------- phase 1: load x, LN1 ----------------

    otp = pool("otp", 1)
    ln1p = pool("ln1p", 1)
    ln1_all = ln1p.tile([P, KC, TKV], F8, name="ln1_all")

    ln_psum = pool("ln_psum", 1, space="PSUM")
    xpool = pool("xpool", 1, side="right")
    x_sb = []
    for c in range(KC):
        xc = xpool.tile([P, TKV], F32, name=f"x_{c}", tag="x", bufs=KC)
        nc.sync.dma_start(xc[:], d["xt"][c * P:(c + 1) * P, :])
        x_sb.append(xc)
    layernorm8(x_sb, TKV, 0, ln1_all, ln_psum)
    close("xpool", "ln_psum")

    # ---------------- phase 2: Q, K projections ----------------

    proj_psum = pool("proj_psum", 1, space="PSUM")
    wpool = pool("wpool1", 1)
    qkv = pool("qkv", 1, side="right")

    Qt = [qkv.tile([P, T], F8, name=f"qt_{mc}", tag="qt", bufs=KC) for mc in range(KC)]
    Kt = [qkv.tile([P, TKV], F8, name=f"kt_{mc}", tag="kt", bufs=KC) for mc in range(KC)]
    # V: kv-chunk t8 at [:, t8, :]; head h at flat offset h*65, ones col at h*65+64
    Vt = qkv.tile([P, 8, VSTRIDE], F8, name="vt")
    O1all = otp.tile([P, KC, T], F8, name="o1all")

    def q_consume(mc, ps):
        nc.vector.tensor_copy(out=Qt[mc][:], in_=ps[:])

    project8(d["wq1"], KP, lambda p8: ln1_all[:, 2 * p8:2 * p8 + 2, 0:T], KC,
             q_consume, wpool, "wq1", proj_psum)

    for thalf in range(2):
        sl = slice(thalf * 512, (thalf + 1) * 512)

        def k_consume(mc, ps, sl=sl):
            nc.vector.tensor_copy(out=Kt[mc][:, sl], in_=ps[:])

        project8(d["wk1"], KP, lambda p8, sl=sl: ln1_all[:, 2 * p8:2 * p8 + 2, sl], KC,
                 k_consume, wpool, "wk1", proj_psum)

    close("proj_psum")

    # ---------------- phase 3: attn1 (V~ projection runs as filler) ----------------

    sc_psum = pool("sc_psum", 1, space="PSUM")
    ov_psum = pool("ov_psum", 1, space="PSUM")
    vp_psum = pool("vp_psum", 1, space="PSUM")
    epool = pool("epool", 3, side="right")

    def vt_head_ap(t8_pair, h):
        # [128, 2, 65] dual-fp8 lhsT: V rows of kv-chunks (2m, 2m+1) for head h
        return Vt[:, t8_pair:t8_pair + 2, h * 65:h * 65 + 65]

    def vproj_filler(nt):
        n0, nsz = ((0, 512), (512, 512), (1024, 256))[nt]

        def run():
            if nt == 0:
                # fill with the ones-column value (16 = weight scale); V evacs
                # overwrite the 64 value columns per head, col 65 stays = WS
                nc.vector.memset(Vt[:], WS)
            wv_sl = []
            for p8 in range(KP):
                wv = wpool.tile([P, 2, 512], F8, name=f"wv1_{nt}_{p8}", tag="wv1", bufs=KP)
                nc.sync.dma_start(wv[:, :, :nsz], d["wv1"][p8][:, :, n0:n0 + nsz])
                wv_sl.append(wv)
            for t8 in range(8):
                ps = vp_psum.tile([P, 512], F32, name=f"psv_{t8}_{n0}", tag="vproj",
                                  bufs=2)
                for p8 in range(KP):
                    nc.tensor.matmul(ps[:, :nsz],
                                     ln1_all[:, 2 * p8:2 * p8 + 2, t8 * P:(t8 + 1) * P],
                                     wv_sl[p8][:, :, :nsz],
                                     start=(p8 == 0), stop=(p8 == KP - 1), perf_mode=DR)
                nc.vector.tensor_copy(
                    out=Vt[:, t8:t8 + 1, n0 // DHEAD * 65:(n0 + nsz) // DHEAD * 65]
                        .rearrange("p t (h e) -> p t h e", e=65)[:, :, :, 0:DHEAD],
                    in_=ps[:, :nsz].rearrange("p (h e) -> p h e", e=DHEAD))
        return run

    # attn1 pipeline: pairs of heads share a score PSUM tile; attnV of an
    # earlier pair interleaves with pair c's score matmuls; fillers emit the V
    # projection to keep the PE warm while exps accumulate.
    depth = 2
    pend = []  # (pair_idx, exp_tile) awaiting attnV
    fillers = (vproj_filler(0), vproj_filler(1), vproj_filler(2))

    def alloc_ov(pc):
        return [ov_psum.tile([DHEAD + 1, 512], F32, name=f"ov{2 * pc + h}",
                             tag="ov", bufs=2) for h in range(2)]

    def av_mm(pc, e_t, ov, m):
        for h in range(2):
            nc.tensor.matmul(ov[h][:], vt_head_ap(2 * m, 2 * pc + h),
                             e_t[:, 2 * m:2 * m + 2, h * 512:(h + 1) * 512],
                             start=(m == 0), stop=(m == 3), perf_mode=DR)

    def finish_pair(pc, ov):
        attn_finish(2 * pc, ov[0], ov_psum, O1all[0:DHEAD, pc, :])
        attn_finish(2 * pc + 1, ov[1], ov_psum, O1all[DHEAD:2 * DHEAD, pc, :])

    for c in range(KC):
        drain = pend.pop(0) if len(pend) >= depth else None
        dov = alloc_ov(drain[0]) if drain else None
        e_t = epool.tile([P, 8, TKV], F8, name=f"exp{c}", tag="exp")
        for k8 in range(8):
            sps = sc_psum.tile([P, 1024], F32, name=f"sps{c}_{k8}", tag="sc", bufs=2)
            for h in range(2):
                nc.tensor.matmul(sps[:, h * 512:(h + 1) * 512],
                                 Kt[c][64 * h:64 * h + 64, k8 * P:(k8 + 1) * P],
                                 Qt[c][64 * h:64 * h + 64, :],
                                 start=True, stop=True, tile_position=(64 * h, 0))
            nc.scalar.activation(e_t[:, k8, :], sps[:], AF.Exp,
                                 scale=SCALE / (WS * WS), bias=ebias[:])
            if drain is not None and k8 < 4:
                av_mm(drain[0], drain[1], dov, k8)
        if drain is not None:
            finish_pair(drain[0], dov)
        if c < len(fillers):
            fillers[c]()
        pend.append((c, e_t))
    for pc, e_t in pend:
        ov = alloc_ov(pc)
        for m in range(4):
            av_mm(pc, e_t, ov, m)
        finish_pair(pc, ov)

    close("epool", "qkv", "vp_psum", "ov_psum", "sc_psum", "wpool1", "ln1p")

    # ---------------- phase 4: out-proj 1 + residual ----------------

    resp = pool("resp", 1)
    wpool = pool("wpool2", 1)
    proj_psum = pool("proj_psum2", 1, space="PSUM")
    x1p = pool("x1p", 1, side="right")
    x1 = [x1p.tile([P, T], F32, name=f"x1_{mc}", tag="x1", bufs=KC) for mc in range(KC)]
    resid = []
    for c in range(KC):
        rc = resp.tile([P, T], F32, name=f"res_{c}", tag="res", bufs=KC)
        nc.sync.dma_start(rc[:], d["xres"][c * P:(c + 1) * P, :])
        resid.append(rc)

    def o1_consume(mc, ps):
        if trivial_bias:
            nc.vector.scalar_tensor_tensor(x1[mc][:], ps[:], 1.0 / WS, resid[mc][:],
                                           mybir.AluOpType.mult, mybir.AluOpType.add)
        else:
            t = tmp.tile([P, T], F32, name=f"o1b_{mc}", tag="o1b", bufs=2)
            nc.scalar.activation(t[:], ps[:], AF.Copy, scale=1.0 / WS, bias=bias_ap(mc))
            nc.vector.tensor_add(x1[mc][:], t[:], resid[mc][:])

    project8(d["wo1"], KP, lambda p8: O1all[:, 2 * p8:2 * p8 + 2, :], KC,
             o1_consume, wpool, "wo1", proj_psum)
    close("wpool2", "resp", "otp", "proj_psum2")

    # ---------------- phase 5: LN2 + attn2 projections ----------------

    o2p = pool("o2p", 1)
    wpool = pool("wpool2b", 1)
    ln_psum = pool("ln_psum2", 1, space="PSUM")
    ln2p = pool("ln2p", 1)
    ln2_all = ln2p.tile([P, KC, T], F8, name="ln2_all")
    layernorm8(x1, T, 1, ln2_all, ln_psum)
    close("ln_psum2")

    proj_psum = pool("proj_psum2b", 1, space="PSUM")
    qkv2 = pool("qkv2", 1, side="right")
    # context, fp8, padded to 80 tokens for the 16B dual-fp8 pair stride
    ctx_all = qkv2.tile([P, KCX, 80], F8, name="ctx_all")
    for c in range(KCX):
        cc = qkv2.tile([P, MCTX], F32, name=f"ctx_{c}", tag="ctx", bufs=2)
        nc.sync.dma_start(cc[:], d["ctxt"][c * P:(c + 1) * P, :])
        nc.any.tensor_copy(out=ctx_all[:, c, 0:MCTX], in_=cc[:])

    Q2t = [qkv2.tile([P, T], F8, name=f"q2t_{mc}", tag="q2t", bufs=KC) for mc in range(KC)]
    K2t = [qkv2.tile([P, MCTX], F8, name=f"k2t_{mc}", tag="k2t", bufs=KC) for mc in range(KC)]
    V2t = qkv2.tile([P, HEADS, DHEAD + 1], F8, name="v2t")
    O2all = o2p.tile([P, KC, T], F8, name="o2all")

    def q2_consume(mc, ps):
        nc.any.tensor_copy(out=Q2t[mc][:], in_=ps[:])

    project8(d["wq2"], KP, lambda p8: ln2_all[:, 2 * p8:2 * p8 + 2, :], KC,
             q2_consume, wpool, "wq2", proj_psum)

    for mc in range(KC):
        wt = wpool.tile([P, KPX, 2, P], F8, name=f"wk2_{mc}", tag="wk2", bufs=3)
        nc.sync.dma_start(wt[:], d["wk2"][mc])
        ps = proj_psum.tile([P, MCTX], F32, name=f"psk2_{mc}", tag="projx", bufs=2)
        for p8 in range(KPX):
            nc.tensor.matmul(ps[:], wt[:, p8], ctx_all[:, 2 * p8:2 * p8 + 2, 0:MCTX],
                             start=(p8 == 0), stop=(p8 == KPX - 1), perf_mode=DR)
        nc.any.tensor_copy(out=K2t[mc][:], in_=ps[:])

    wv2_sb = []
    for p8 in range(KPX):
        wv = wpool.tile([P, 2, DIM], F8, name=f"wv2_{p8}", tag="wv2", bufs=KPX)
        nc.sync.dma_start(wv[:], d["wv2"][p8])
        wv2_sb.append(wv)

    close("ln2p", "proj_psum2b")

    # ---------------- phase 6: attn2 ----------------

    sc_psum = pool("sc_psum2", 1, space="PSUM")
    ov_psum = pool("ov_psum2", 1, space="PSUM")
    vp_psum = pool("vp_psum2", 1, space="PSUM")
    epool = pool("epool2", 6, side="right")

    def v2proj_filler():
        nc.vector.memset(V2t[:], WS)
        for n0, nsz in ((0, 512), (512, 512), (1024, 256)):
            ps = vp_psum.tile([MCTX, 512], F32, name=f"psv2_{n0}", tag="vproj", bufs=2)
            for p8 in range(KPX):
                nc.tensor.matmul(ps[:, :nsz], ctx_all[:, 2 * p8:2 * p8 + 2, 0:MCTX],
                                 wv2_sb[p8][:, :, n0:n0 + nsz],
                                 start=(p8 == 0), stop=(p8 == KPX - 1), perf_mode=DR)
            nc.any.tensor_copy(
                out=V2t[:MCTX, n0 // DHEAD:(n0 + nsz) // DHEAD, 0:DHEAD],
                in_=ps[:, :nsz].rearrange("p (h e) -> p h e", e=DHEAD))

    # attn2 pipeline (kv=77, single chunk, plain fp8 matmuls)
    pend2 = []
    for c in range(KC):
        drain = pend2.pop(0) if len(pend2) >= 2 else None
        if drain is not None:
            dov = alloc_ov2 = [ov_psum.tile([DHEAD + 1, 512], F32, name=f"ov2_{2 * drain[0] + h}",
                                            tag="ov", bufs=2) for h in range(2)]
            for h in range(2):
                nc.tensor.matmul(dov[h][:], V2t[:MCTX, 2 * drain[0] + h, :],
                                 drain[1][:MCTX, h * 512:(h + 1) * 512],
                                 start=True, stop=True)
            attn_finish(40 + 2 * drain[0], dov[0], ov_psum, O2all[0:DHEAD, drain[0], :], True)
            attn_finish(41 + 2 * drain[0], dov[1], ov_psum, O2all[DHEAD:2 * DHEAD, drain[0], :], True)
        sps = sc_psum.tile([MCTX, 1024], F32, name=f"sps2_{c}", tag="sc", bufs=2)
        for h in range(2):
            nc.tensor.matmul(sps[:, h * 512:(h + 1) * 512],
                             K2t[c][64 * h:64 * h + 64, :],
                             Q2t[c][64 * h:64 * h + 64, :],
                             start=True, stop=True, tile_position=(64 * h, 0))
        e_t = epool.tile([MCTX, 1024], F8, name=f"exp2_{c}", tag="exp2")
        nc.scalar.activation(e_t[:], sps[:], AF.Exp, scale=SCALE / (WS * WS),
                             bias=ebias[:MCTX])
        if c == 0:
            v2proj_filler()
        pend2.append((c, e_t))
    for pc, e_t in pend2:
        ov = [ov_psum.tile([DHEAD + 1, 512], F32, name=f"ov2t_{2 * pc + h}",
                           tag="ov", bufs=2) for h in range(2)]
        for h in range(2):
            nc.tensor.matmul(ov[h][:], V2t[:MCTX, 2 * pc + h, :],
                             e_t[:MCTX, h * 512:(h + 1) * 512],
                             start=True, stop=True)
        attn_finish(40 + 2 * pc, ov[0], ov_psum, O2all[0:DHEAD, pc, :], True)
        attn_finish(41 + 2 * pc, ov[1], ov_psum, O2all[DHEAD:2 * DHEAD, pc, :], True)

    close("epool2", "qkv2", "vp_psum2", "ov_psum2", "sc_psum2", "wpool2b")

    # ---------------- phase 7: out-proj 2 + residual ----------------

    x2p = pool("x2p", 1)
    wpool = pool("wpool3", 1)
    proj_psum = pool("proj_psum3", 1, space="PSUM")
    x2 = [x2p.tile([P, T], F32, name=f"x2_{mc}", tag="x2", bufs=KC) for mc in range(KC)]

    def o2_consume(mc, ps):
        if trivial_bias:
            nc.vector.scalar_tensor_tensor(x2[mc][:], ps[:], 1.0 / WS, x1[mc][:],
                                           mybir.AluOpType.mult, mybir.AluOpType.add)
        else:
            t = tmp.tile([P, T], F32, name=f"o2b_{mc}", tag="o1b", bufs=2)
            nc.scalar.activation(t[:], ps[:], AF.Copy, scale=1.0 / WS, bias=bias_ap(10 + mc))
            nc.vector.tensor_add(x2[mc][:], t[:], x1[mc][:])

    project8(d["wo2"], KP, lambda p8: O2all[:, 2 * p8:2 * p8 + 2, :], KC,
             o2_consume, wpool, "wo2", proj_psum)
    close("wpool3", "x1p", "proj_psum3")

    # ---------------- phase 8: LN3 + GEGLU FF (fp16) ----------------

    hhp = pool("hhp", 1)
    hht = [hhp.tile([P, T], F16, name=f"hh_{j}", tag="hh", bufs=JFF) for j in range(JFF)]

    ln_psum = pool("ln_psum3", 1, space="PSUM")
    ln3p = pool("ln3p", 1)
    ln3t = [ln3p.tile([P, T], F16, name=f"ln3_{c}", tag="ln3", bufs=KC) for c in range(KC)]
    layernorm16(x2, T, 2, ln3t, ln_psum)
    close("ln_psum3")

    wpool = pool("wpool4a", 1)
    proj_psum = pool("proj_psum4", 1, space="PSUM")
    for j in range(JFF):
        wg = wpool.tile([P, KC, P], F16, name=f"wg_{j}", tag="wff1g", bufs=3)
        nc.sync.dma_start(wg[:], d["wff1"][JFF + j])
        gps = proj_psum.tile([P, 512], F32, name=f"gps_{j}", tag="proj", bufs=4)
        for kc in range(KC):
            nc.tensor.matmul(gps[:], wg[:, kc], ln3t[kc][:], start=(kc == 0),
                             stop=(kc == KC - 1))
        gel = tmp.tile([P, T], F16, name=f"gel_{j}", tag="gel", bufs=3)
        if trivial_bias:
            nc.scalar.activation(gel[:], gps[:], AF.Gelu_apprx_tanh)
        else:
            nc.scalar.activation(gel[:], gps[:], AF.Gelu_apprx_tanh, bias=bias_ap(60 + j))

        wa = wpool.tile([P, KC, P], F16, name=f"wa_{j}", tag="wff1a", bufs=3)
        nc.sync.dma_start(wa[:], d["wff1"][j])
        aps = proj_psum.tile([P, 512], F32, name=f"aps_{j}", tag="proj", bufs=4)
        for kc in range(KC):
            nc.tensor.matmul(aps[:], wa[:, kc], ln3t[kc][:], start=(kc == 0),
                             stop=(kc == KC - 1))
        if trivial_bias:
            nc.vector.tensor_mul(hht[j][:], aps[:], gel[:])
        else:
            nc.vector.scalar_tensor_tensor(hht[j][:], aps[:], bias_ap(20 + j), gel[:],
                                           mybir.AluOpType.add, mybir.AluOpType.mult)

    close("wpool4a", "ln3p")

    # ---------------- phase 9: FF down-proj + residual -> out ----------------

    wpool = pool("wpool4b", 1)
    outp = pool("outp", 4)
    for mc in range(KC):
        wt = wpool.tile([P, JFF, P], F16, name=f"wff2_{mc}", tag="wff2", bufs=2)
        nc.sync.dma_start(wt[:], d["wff2"][mc])
        ps = proj_psum.tile([P, 512], F32, name=f"psf2_{mc}", tag="proj", bufs=4)
        for kc in range(JFF):
            nc.tensor.matmul(ps[:], wt[:, kc], hht[kc][:], start=(kc == 0),
                             stop=(kc == JFF - 1))
        ot = outp.tile([P, T], F32, name=f"out_{mc}", tag="out")
        if trivial_bias:
            nc.vector.tensor_add(ot[:], ps[:], x2[mc][:])
        else:
            nc.vector.scalar_tensor_tensor(ot[:], ps[:], bias_ap(100 + mc), x2[mc][:],
                                           mybir.AluOpType.add, mybir.AluOpType.add)
        nc.sync.dma_start(d["out"][mc * P:(mc + 1) * P, :], ot[:])

    close("outp", "wpool4b", "hhp", "x2p", "o2p", "tmp", "const", "proj_psum4")


def _lhst_layout(w, n_kc, n_mc):
    """[K, M] f32 -> fp16 [n_mc, 128, n_kc, 128] so block [mc] is the
    contiguous stationary-operand group for output chunk mc."""
    return np.ascontiguousarray(
        w.reshape(n_kc, P, n_mc, P).transpose(2, 1, 0, 3).astype(np.float16))


def _lhst8_layout(w, n_kc, n_mc):
    """[K, M] f32 -> fp8 x16 [n_mc, 128, n_kc//2, 2, 128]: dual-fp8 stationary
    pairs over adjacent k-chunks."""
    a = (np.asarray(w, np.float32) * WS).reshape(n_kc // 2, 2, P, n_mc, P)
    return np.ascontiguousarray(a.transpose(3, 2, 0, 1, 4)).astype(E4)


def _rhs8_layout(w, n_kc):
    """[K, M] f32 -> fp8 x16 [n_kc//2, 128, 2, M]: dual-fp8 moving pairs."""
    a = (np.asarray(w, np.float32) * WS).reshape(n_kc // 2, 2, P, -1)
    return np.ascontiguousarray(a.transpose(0, 2, 1, 3)).astype(E4)


_BUILT = {}


def _build(trivial_aff, trivial_bias):
    key = (trivial_aff, trivial_bias)
    if key in _BUILT:
        return _BUILT[key]
    nc = bacc.Bacc("TRN2", target_bir_lowering=False, debug=False, num_devices=N_CORES)
    d = {
        "xt": nc.dram_tensor("xt", [DIM, TKV], F32, kind="ExternalInput").ap(),
        "ctxt": nc.dram_tensor("ctxt", [CTX_DIM, MCTX], F32, kind="ExternalInput").ap(),
        "xres": nc.dram_tensor("xres", [DIM, T], F32, kind="ExternalInput").ap(),
        "wq1": nc.dram_tensor("wq1", [KC, P, KP, 2, P], F8, kind="ExternalInput").ap(),
        "wk1": nc.dram_tensor("wk1", [KC, P, KP, 2, P], F8, kind="ExternalInput").ap(),
        "wv1": nc.dram_tensor("wv1", [KP, P, 2, DIM], F8, kind="ExternalInput").ap(),
        "wo1": nc.dram_tensor("wo1", [KC, P, KP, 2, P], F8, kind="ExternalInput").ap(),
        "wq2": nc.dram_tensor("wq2", [KC, P, KP, 2, P], F8, kind="ExternalInput").ap(),
        "wk2": nc.dram_tensor("wk2", [KC, P, KPX, 2, P], F8, kind="ExternalInput").ap(),
        "wv2": nc.dram_tensor("wv2", [KPX, P, 2, DIM], F8, kind="ExternalInput").ap(),
        "wo2": nc.dram_tensor("wo2", [KC, P, KP, 2, P], F8, kind="ExternalInput").ap(),
        "wff1": nc.dram_tensor("wff1", [2 * JFF, P, KC, P], F16, kind="ExternalInput").ap(),
        "wff2": nc.dram_tensor("wff2", [KC, P, JFF, P], F16, kind="ExternalInput").ap(),
        "out": nc.dram_tensor("out", [DIM, T], F32, kind="ExternalOutput").ap(),
    }
    if not trivial_aff:
        d["aff"] = nc.dram_tensor("aff", [P, 60], F32, kind="ExternalInput").ap()
    if not trivial_bias:
        d["biases"] = nc.dram_tensor("biases", [P, 110], F32, kind="ExternalInput").ap()
    with tile.TileContext(nc) as tc:
        _emit(tc, d, trivial_aff, trivial_bias)
    nc.compile()
    _BUILT[key] = nc
    return nc


def kernel(x, context,
           g1, be1, wq1, wk1, wv1, wo1, bo1,
           g2, be2, wq2, wk2, wv2, wo2, bo2,
           g3, be3, w_ff1, b_ff1, w_ff2, b_ff2,
           _trace=False):
    global last_exec_time_ns
    x = np.asarray(x, np.float32)
    context = np.asarray(context, np.float32)

    affs = [np.asarray(a, np.float32) for a in (g1, be1, g2, be2, g3, be3)]
    biases = [np.asarray(b, np.float32) for b in (bo1, bo2, b_ff1, b_ff2)]
    trivial_aff = all(np.all(a == (1.0 if i % 2 == 0 else 0.0))
                      for i, a in enumerate(affs))
    trivial_bias = all(np.all(b == 0.0) for b in biases)

    nc = _build(trivial_aff, trivial_bias)

    shared = {
        "wq1": _lhst8_layout(np.asarray(wq1, np.float32), KC, KC),
        "wk1": _lhst8_layout(np.asarray(wk1, np.float32), KC, KC),
        "wv1": _rhs8_layout(np.asarray(wv1, np.float32), KC),
        "wo1": _lhst8_layout(np.asarray(wo1, np.float32), KC, KC),
        "wq2": _lhst8_layout(np.asarray(wq2, np.float32), KC, KC),
        "wk2": _lhst8_layout(np.asarray(wk2, np.float32), KCX, KC),
        "wv2": _rhs8_layout(np.asarray(wv2, np.float32), KCX),
        "wo2": _lhst8_layout(np.asarray(wo2, np.float32), KC, KC),
        "wff1": _lhst_layout(np.asarray(w_ff1, np.float32), KC, 2 * JFF),
        "wff2": _lhst_layout(np.asarray(w_ff2, np.float32), JFF, KC),
    }
    if not trivial_aff:
        aff = np.zeros([P, 60], np.float32)
        for i, a in enumerate(affs):
            # col = ln_idx*20 + (0 for g / 10 for be) + chunk
            ln_idx, j = i // 2, i % 2
            aff[:, ln_idx * 20 + j * 10: ln_idx * 20 + j * 10 + 10] = \
                a.reshape(KC, P).T
        shared["aff"] = aff
    if not trivial_bias:
        bb = np.zeros([P, 110], np.float32)
        bb[:, 0:10] = biases[0].reshape(KC, P).T
        bb[:, 10:20] = biases[1].reshape(KC, P).T
        bb[:, 20:100] = biases[2].reshape(2 * JFF, P).T
        bb[:, 100:110] = biases[3].reshape(KC, P).T
        shared["biases"] = bb

    in_maps = []
    for b in range(BATCH):
        ctxt = np.ascontiguousarray(context[b].T)
        for h in range(2):
            xr = np.roll(x[b], -h * T, axis=0)
            m = dict(shared)
            xrt = np.ascontiguousarray(xr.T)
            m["xt"] = xrt
            m["xres"] = np.ascontiguousarray(xrt[:, 0:T])
            m["ctxt"] = ctxt
            in_maps.append(m)

    res = bass_utils.run_bass_kernel_spmd(
        nc, in_maps, core_ids=list(range(N_CORES)), trace=_trace)
    last_exec_time_ns = res.exec_time_ns

    out = np.empty((BATCH, NTOK, DIM), np.float32)
    for b in range(BATCH):
        for h in range(2):
            out[b, h * T:(h + 1) * T, :] = res.results[b * 2 + h]["out"].T
    return out


# revision 10
# speedup vs baseline: 1.1359x; 1.0241x over previous
"""BasicTransformerBlock on 8 TRN2 NeuronCores.

Sharding: data-parallel, core = (batch b in 0..3) x (sequence half h in 0..1).
Each core receives its batch element's full sequence rotated so its local 512
rows come first (softmax over keys is permutation invariant), computes K/V of
attn1 for all 1024 tokens (duplicated across the pair, ~10% extra FLOPs, zero
collectives), and everything else for its 512 local tokens only.

On-chip layout: feature-major activations [features on partitions, tokens on
free axis] so every projection consumes natural-layout weights as the matmul
stationary operand.

Precision: the attention path (Q/K/V/O projections, attnV) runs in fp8 e4m3
with DoubleRow dual-fp8 matmuls (2x PE throughput). Weights are pre-scaled by
16 on the host so they sit in e4m3's normal range; Q/K/V keep the x16 scale in
their fp8 tiles (scores come out x256, folded into the exp scale; the V ones-
column is 16 so the denominator cancels it). The GEGLU FF stays fp16 (fp8
there costs ~1.5e-2 relative error, over budget). The residual stream, LN
math and PSUM accumulation stay fp32. LayerNorm partition reductions use
dual-fp8 ones-matmuls; per-token broadcasts use fp16 ones-matmuls; softmax
denominators come free from a ones-column appended to V.

Dual-fp8 DoubleRow matmuls pair two 128-deep contractions per instruction;
hardware requires the pair stride in both operands to be a multiple of 16
bytes and PSUM outputs to start at partition 0 (hence the 1312-wide padded V
tile and the 80-wide padded context tile).
"""

import sys
import types

sys.path.insert(0, "/opt/trn_rl_repo")

# concourse fetches the NTFF profile hook from antenv.axon_hooks, which the
# agent image's antenv stub lacks. Register a shim so trace=True works.
if "antenv.axon_hooks" not in sys.modules:
    _hooks = types.ModuleType("antenv.axon_hooks")
    _HOOK = [None]

    def _get_hook():
        if _HOOK[0] is None:
            try:
                from trn_agent_boot.trn_boot import _ntff_profile_via_ctypes

                _HOOK[0] = _ntff_profile_via_ctypes("/opt/axon/libaxon_pjrt.so")
            except Exception:
                _HOOK[0] = None
        return _HOOK[0]

    _hooks.get_axon_ntff_profile_hook = _get_hook
    _hooks.set_axon_ntff_profile_hook = lambda h: _HOOK.__setitem__(0, h)
    sys.modules["antenv.axon_hooks"] = _hooks
    try:
        import antenv

        antenv.axon_hooks = _hooks
    except ImportError:
        pass

import ml_dtypes
import numpy as np

import concourse.bass as bass
import concourse.mybir as mybir
import concourse.tile as tile
from concourse import bacc, bass_utils

dt = mybir.dt
F32, F16, F8 = dt.float32, dt.float16, dt.float8e4
AF = mybir.ActivationFunctionType
DR = mybir.MatmulPerfMode.DoubleRow
E4 = ml_dtypes.float8_e4m3

DIM, HEADS, DHEAD, CTX_DIM, DFF = 1280, 20, 64, 768, 5120
BATCH, NTOK, MCTX = 4, 1024, 77
EPS = 1e-5
SCALE = DHEAD ** -0.5
N_CORES = 8
T = 512         # local tokens per core
TKV = 1024      # attn1 key/value tokens per core
KC = DIM // 128           # 10
KP = KC // 2              # 5 dual-fp8 k-chunk pairs
KCX = CTX_DIM // 128      # 6
KPX = KCX // 2            # 3
JFF = DFF // 128          # 40 (chunks of the gated hidden)
P = 128
WS = 16.0                 # host-side fp8 weight scale
EBIAS = -1.0              # exp bias (cancels in softmax, keeps exps in range)
VSTRIDE = 1312            # padded 20*(DHEAD+1)=1300 -> 16B-aligned pair stride

last_exec_time_ns = None


def _emit(tc, d, trivial_aff, trivial_bias):
    nc = tc.nc
    pools = {}

    def pool(name, bufs, space="SBUF", side="left"):
        p = tc.alloc_tile_pool(name=name, bufs=bufs, space=space, side=side)
        pools[name] = p
        return p

    def close(*names):
        for n in names:
            pools.pop(n).release()

    # Pools are two LIFO stacks (left/right) per memory space; lifetimes below
    # are arranged so every release pops the top of its stack.
    const = pool("const", 1)
    ones_col = const.tile([P, 1], F16, name="ones_col")
    nc.vector.memset(ones_col[:], 1.0)
    ones_row = const.tile([1, P], F16, name="ones_row")
    nc.vector.memset(ones_row[:], 1.0)
    # dual-fp8 ones pair for LN stat reductions (16B-aligned pair stride)
    ones8 = const.tile([P, 2, 16], F8, name="ones8")
    nc.vector.memset(ones8[:], 1.0)
    ebias = const.tile([P, 1], F32, name="ebias")
    nc.vector.memset(ebias[:], EBIAS)
    if not trivial_aff:
        aff = const.tile([P, 60], F32, name="aff")
        nc.sync.dma_start(aff[:], d["aff"])
    if not trivial_bias:
        biases = const.tile([P, 110], F32, name="biases")
        nc.sync.dma_start(biases[:], d["biases"])

    tmp = pool("tmp", 1)

    def bias_ap(col):
        return biases[:, col:col + 1]

    # ---------------- LN helpers (fp8 path, split for pipelining) ----------

    def ln8_begin(ln_idx, t, ln_psum):
        st = {
            "sums": ln_psum.tile([1, 512], F32, name=f"lns{ln_idx}_{t}", tag="lnstat", bufs=2),
            "sq": ln_psum.tile([1, 512], F32, name=f"lnq{ln_idx}_{t}", tag="lnstat", bufs=2),
            "xh8": tmp.tile([P, KC, 512], F8, name=f"xh8_{ln_idx}_{t}", tag="xh8", bufs=2),
            "id": (ln_idx, t),
        }
        return st

    def ln8_chunk_pair(st, p8, x_fn):
        """Feed chunks (2p8, 2p8+1); x_fn(c) -> f32 AP [128, 512]."""
        ln_idx, t = st["id"]
        xh8 = st["xh8"]
        xsq = tmp.tile([P, 2, 512], F8, name=f"xsq{ln_idx}_{t}_{p8}", tag="xsq", bufs=3)
        for s in range(2):
            c = 2 * p8 + s
            nc.scalar.copy(xh8[:, c, :], x_fn(c))
            nc.gpsimd.tensor_mul(xsq[:, s, :], xh8[:, c, :], xh8[:, c, :])
        nc.tensor.matmul(st["sums"][:], ones8[:, :, 0:1], xh8[:, 2 * p8:2 * p8 + 2, :],
                         start=(p8 == 0), stop=(p8 == KP - 1), perf_mode=DR)
        nc.tensor.matmul(st["sq"][:], ones8[:, :, 0:1], xsq[:],
                         start=(p8 == 0), stop=(p8 == KP - 1), perf_mode=DR)

    def ln8_finish(st, out_big, sl, ln_psum):
        ln_idx, t = st["id"]
        xh8 = st["xh8"]
        ssum = tmp.tile([1, 512], F16, name=f"ssum{ln_idx}_{t}", tag="ssum", bufs=2)
        nc.scalar.copy(ssum[:], st["sums"][:])
        ssq = tmp.tile([1, 512], F16, name=f"ssq{ln_idx}_{t}", tag="ssq", bufs=2)
        nc.scalar.copy(ssq[:], st["sq"][:])
        bs_ps = ln_psum.tile([P, 512], F32, name=f"bs{ln_idx}_{t}", tag="lnbc", bufs=2)
        nc.tensor.matmul(bs_ps[:], ones_row[:], ssum[:], start=True, stop=True)
        bq_ps = ln_psum.tile([P, 512], F32, name=f"bq{ln_idx}_{t}", tag="lnbc", bufs=2)
        nc.tensor.matmul(bq_ps[:], ones_row[:], ssq[:], start=True, stop=True)
        mu = tmp.tile([P, 512], F32, name=f"mu{ln_idx}_{t}", tag="mu", bufs=2)
        nc.vector.tensor_scalar_mul(mu[:], bs_ps[:], 1.0 / DIM)
        musq = tmp.tile([P, 512], F32, name=f"musq{ln_idx}_{t}", tag="musq", bufs=1)
        nc.vector.tensor_mul(musq[:], mu[:], mu[:])
        # musq - EPS, so var = ex2 - musq + EPS below
        nc.vector.tensor_scalar_sub(musq[:], musq[:], EPS)
        var = tmp.tile([P, 512], F32, name=f"var{ln_idx}_{t}", tag="var", bufs=1)
        nc.vector.scalar_tensor_tensor(var[:], bq_ps[:], 1.0 / DIM, musq[:],
                                       mybir.AluOpType.mult, mybir.AluOpType.subtract)
        std = tmp.tile([P, 512], F32, name=f"std{ln_idx}_{t}", tag="std", bufs=1)
        nc.scalar.sqrt(std[:], var[:])
        rstd = tmp.tile([P, 512], F32, name=f"rstd{ln_idx}_{t}", tag="rstd", bufs=2)
        nc.vector.reciprocal_approx_fast(rstd[:], std[:])
        rstd16 = tmp.tile([P, 512], F16, name=f"rstd16{ln_idx}_{t}", tag="rstd16", bufs=2)
        nc.vector.tensor_copy(out=rstd16[:], in_=rstd[:])
        mu16 = tmp.tile([P, 512], F16, name=f"mu16{ln_idx}_{t}", tag="mu16", bufs=2)
        nc.vector.tensor_copy(out=mu16[:], in_=mu[:])
        for c in range(KC):
            xm = tmp.tile([P, 512], F16, name=f"xm{ln_idx}_{t}_{c}", tag="xm", bufs=3)
            nc.vector.tensor_sub(xm[:], xh8[:, c, :], mu16[:])
            if trivial_aff:
                nc.vector.tensor_mul(out_big[:, c, sl], xm[:], rstd16[:])
            else:
                xn = tmp.tile([P, 512], F16, name=f"xn{ln_idx}_{t}_{c}", tag="xn", bufs=3)
                nc.vector.tensor_mul(xn[:], xm[:], rstd16[:])
                g_ap = aff[:, ln_idx * 20 + c: ln_idx * 20 + c + 1]
                be_ap = aff[:, ln_idx * 20 + 10 + c: ln_idx * 20 + 10 + c + 1]
                xg = tmp.tile([P, 512], F16, name=f"xg{ln_idx}_{t}_{c}", tag="xg", bufs=3)
                nc.vector.tensor_scalar_mul(xg[:], xn[:], g_ap)
                nc.scalar.activation(out_big[:, c, sl], xg[:], AF.Copy, bias=be_ap)

    # fp16 LN (before the fp16 FF): fp16 stats, fp16 out, 2x DVE normalize
    def ln16_begin(ln_idx, ln_psum):
        return {
            "sums": ln_psum.tile([1, 512], F32, name=f"lns{ln_idx}", tag="lnstat", bufs=2),
            "sq": ln_psum.tile([1, 512], F32, name=f"lnq{ln_idx}", tag="lnstat", bufs=2),
            "xhs": [],
            "id": ln_idx,
        }

    def ln16_chunk(st, c, x_ap):
        ln_idx = st["id"]
        xh = tmp.tile([P, 512], F16, name=f"xh{ln_idx}_{c}", tag="xh", bufs=10)
        nc.scalar.copy(xh[:], x_ap)
        st["xhs"].append(xh)
        xsq = tmp.tile([P, 512], F16, name=f"xsqf{ln_idx}_{c}", tag="xsqf", bufs=3)
        nc.gpsimd.tensor_mul(xsq[:], xh[:], xh[:])
        nc.tensor.matmul(st["sums"][:], ones_col[:], xh[:],
                         start=(c == 0), stop=(c == KC - 1))
        nc.tensor.matmul(st["sq"][:], ones_col[:], xsq[:],
                         start=(c == 0), stop=(c == KC - 1))

    def ln16_finish(st, out_tiles, ln_psum):
        ln_idx = st["id"]
        ssum = tmp.tile([1, 512], F16, name=f"ssum{ln_idx}", tag="ssum", bufs=2)
        nc.scalar.copy(ssum[:], st["sums"][:])
        ssq = tmp.tile([1, 512], F16, name=f"ssq{ln_idx}", tag="ssq", bufs=2)
        nc.scalar.copy(ssq[:], st["sq"][:])
        bs_ps = ln_psum.tile([P, 512], F32, name=f"bs{ln_idx}", tag="lnbc", bufs=2)
        nc.tensor.matmul(bs_ps[:], ones_row[:], ssum[:], start=True, stop=True)
        bq_ps = ln_psum.tile([P, 512], F32, name=f"bq{ln_idx}", tag="lnbc", bufs=2)
        nc.tensor.matmul(bq_ps[:], ones_row[:], ssq[:], start=True, stop=True)
        mu = tmp.tile([P, 512], F32, name=f"mu{ln_idx}", tag="mu", bufs=2)
        nc.vector.tensor_scalar_mul(mu[:], bs_ps[:], 1.0 / DIM)
        musq = tmp.tile([P, 512], F32, name=f"musq{ln_idx}", tag="musq", bufs=1)
        nc.vector.tensor_mul(musq[:], mu[:], mu[:])
        nc.vector.tensor_scalar_sub(musq[:], musq[:], EPS)
        var = tmp.tile([P, 512], F32, name=f"var{ln_idx}", tag="var", bufs=1)
        nc.vector.scalar_tensor_tensor(var[:], bq_ps[:], 1.0 / DIM, musq[:],
                                       mybir.AluOpType.mult, mybir.AluOpType.subtract)
        std = tmp.tile([P, 512], F32, name=f"std{ln_idx}", tag="std", bufs=1)
        nc.scalar.sqrt(std[:], var[:])
        rstd = tmp.tile([P, 512], F32, name=f"rstd{ln_idx}", tag="rstd", bufs=2)
        nc.vector.reciprocal_approx_fast(rstd[:], std[:])
        rstd16 = tmp.tile([P, 512], F16, name=f"rstd16{ln_idx}", tag="rstd16", bufs=2)
        nc.vector.tensor_copy(out=rstd16[:], in_=rstd[:])
        mu16 = tmp.tile([P, 512], F16, name=f"mu16{ln_idx}", tag="mu16", bufs=2)
        nc.vector.tensor_copy(out=mu16[:], in_=mu[:])
        for c in range(KC):
            xm = tmp.tile([P, 512], F16, name=f"xm{ln_idx}_{c}", tag="xm", bufs=3)
            nc.vector.tensor_sub(xm[:], st["xhs"][c][:], mu16[:])
            if trivial_aff:
                nc.vector.tensor_mul(out_tiles[c][:], xm[:], rstd16[:])
            else:
                xn = tmp.tile([P, 512], F16, name=f"xn{ln_idx}_{c}", tag="xn", bufs=3)
                nc.vector.tensor_mul(xn[:], xm[:], rstd16[:])
                g_ap = aff[:, ln_idx * 20 + c: ln_idx * 20 + c + 1]
                be_ap = aff[:, ln_idx * 20 + 10 + c: ln_idx * 20 + 10 + c + 1]
                xg = tmp.tile([P, 512], F16, name=f"xg{ln_idx}_{c}", tag="xg", bufs=3)
                nc.vector.tensor_scalar_mul(xg[:], xn[:], g_ap)
                nc.scalar.activation(out_tiles[c][:], xg[:], AF.Copy, bias=be_ap)

    def attn_finish(head, ops_, ov_psum, out_ap):
        usb = tmp.tile([DHEAD + 1, 512], F16, name=f"usb{head}", tag="usb", bufs=4)
        nc.vector.tensor_copy(out=usb[:], in_=ops_[:])
        den = tmp.tile([1, 512], F32, name=f"den{head}", tag="den", bufs=2)
        nc.vector.tensor_copy(out=den[:], in_=usb[DHEAD:DHEAD + 1, :])
        rec32 = tmp.tile([1, 512], F32, name=f"rec32_{head}", tag="rec32", bufs=2)
        nc.vector.reciprocal_approx_fast(rec32[:], den[:])
        rec = tmp.tile([1, 512], F16, name=f"rec{head}", tag="rec", bufs=2)
        nc.vector.tensor_copy(out=rec[:], in_=rec32[:])
        bps = ov_psum.tile([DHEAD, 512], F32, name=f"bps{head}", tag="ov", bufs=2)
        nc.tensor.matmul(bps[:], ones_row[:, :DHEAD], rec[:],
                         start=True, stop=True)
        nc.vector.tensor_mul(out_ap, usb[:DHEAD, :], bps[:])

    def project8(w_d, n_kp, rhs_fn, n_mc, consume, wpool, wtag, psum_p, wbufs=3,
                 after=None, pbufs=4):
        """out[mc] = sum_p dualfp8( w[mc][:,p] , rhs(p) ); consume(mc, psum)."""
        for mc in range(n_mc):
            wt = wpool.tile([P, n_kp, 2, P], F8, name=f"{wtag}_{mc}", tag=wtag, bufs=wbufs)
            nc.sync.dma_start(wt[:], w_d[mc])
            ps = psum_p.tile([P, 512], F32, name=f"ps_{wtag}_{mc}", tag="proj", bufs=pbufs)
            for p8 in range(n_kp):
                nc.tensor.matmul(ps[:], wt[:, p8], rhs_fn(p8),
                                 start=(p8 == 0), stop=(p8 == n_kp - 1), perf_mode=DR)
            consume(mc, ps)
            if after is not None:
                after(mc)

    # ================ phase A: load x, LN1, Q/K proj, attn1 ================

    otp = pool("otp", 1)
    ln1p = pool("ln1p", 1)
    ln1_all = ln1p.tile([P, KC, TKV], F8, name="ln1_all")
    wpool = pool("wpool1", 1)
    psumA = pool("psumA", 1, space="PSUM")
    xpool = pool("xpool", 1)

    x_sb = []
    for c in range(KC):
        xc = xpool.tile([P, TKV], F32, name=f"x_{c}", tag="x", bufs=KC)
        nc.sync.dma_start(xc[:], d["xt"][c * P:(c + 1) * P, :])
        x_sb.append(xc)

    # LN1 block t=0 (tokens 0..511)
    st0 = ln8_begin(0, 0, psumA)
    for p8 in range(KP):
        ln8_chunk_pair(st0, p8, lambda c: x_sb[c][:, 0:512])
    ln8_finish(st0, ln1_all, slice(0, 512), psumA)

    qkv = pool("qkv", 1, side="right")
    Qt = [qkv.tile([P, T], F8, name=f"qt_{mc}", tag="qt", bufs=KC) for mc in range(KC)]
    Kt = [qkv.tile([P, TKV], F8, name=f"kt_{mc}", tag="kt", bufs=KC) for mc in range(KC)]
    # V: kv-chunk t8 at [:, t8, :]; head h at flat offset h*65, ones col at h*65+64
    Vt = qkv.tile([P, 8, VSTRIDE], F8, name="vt")
    O1all = otp.tile([P, KC, T], F8, name="o1all")

    def q_consume(mc, ps):
        nc.vector.tensor_copy(out=Qt[mc][:], in_=ps[:])

    project8(d["wq1"], KP, lambda p8: ln1_all[:, 2 * p8:2 * p8 + 2, 0:T], KC,
             q_consume, wpool, "wq1", psumA)

    # LN1 block t=1 runs on ACT/DVE/gpsimd while the PE does Q/K halves
    st1 = ln8_begin(0, 1, psumA)
    for p8 in range(KP):
        ln8_chunk_pair(st1, p8, lambda c: x_sb[c][:, 512:1024])
    ln8_finish(st1, ln1_all, slice(512, 1024), psumA)

    for thalf in range(2):
        sl = slice(thalf * 512, (thalf + 1) * 512)

        def k_consume(mc, ps, sl=sl):
            nc.vector.tensor_copy(out=Kt[mc][:, sl], in_=ps[:])

        project8(d["wk1"], KP, lambda p8, sl=sl: ln1_all[:, 2 * p8:2 * p8 + 2, sl], KC,
                 k_consume, wpool, "wk1", psumA)

    close("xpool", "psumA")

    # ---------------- attn1 (V~ projection runs as filler) ----------------

    sc_psum = pool("sc_psum", 1, space="PSUM")
    ov_psum = pool("ov_psum", 1, space="PSUM")
    vp_psum = pool("vp_psum", 1, space="PSUM")
    epool = pool("epool", 3, side="right")

    def vt_head_ap(t8_pair, h):
        # [128, 2, 65] dual-fp8 lhsT: V rows of kv-chunks (2m, 2m+1) for head h
        return Vt[:, t8_pair:t8_pair + 2, h * 65:h * 65 + 65]

    def vproj_filler(nt):
        n0, nsz = ((0, 512), (512, 512), (1024, 256))[nt]

        def run():
            if nt == 0:
                # fill with the ones-column value (16 = weight scale); V evacs
                # overwrite the 64 value columns per head, col 65 stays = WS
                nc.vector.memset(Vt[:], WS)
            wv_sl = []
            for p8 in range(KP):
                wv = wpool.tile([P, 2, 512], F8, name=f"wv1_{nt}_{p8}", tag="wv1", bufs=KP)
                nc.sync.dma_start(wv[:, :, :nsz], d["wv1"][p8][:, :, n0:n0 + nsz])
                wv_sl.append(wv)
            for t8 in range(8):
                ps = vp_psum.tile([P, 512], F32, name=f"psv_{t8}_{n0}", tag="vproj",
                                  bufs=2)
                for p8 in range(KP):
                    nc.tensor.matmul(ps[:, :nsz],
                                     ln1_all[:, 2 * p8:2 * p8 + 2, t8 * P:(t8 + 1) * P],
                                     wv_sl[p8][:, :, :nsz],
                                     start=(p8 == 0), stop=(p8 == KP - 1), perf_mode=DR)
                nc.vector.tensor_copy(
                    out=Vt[:, t8:t8 + 1, n0 // DHEAD * 65:(n0 + nsz) // DHEAD * 65]
                        .rearrange("p t (h e) -> p t h e", e=65)[:, :, :, 0:DHEAD],
                    in_=ps[:, :nsz].rearrange("p (h e) -> p h e", e=DHEAD))
        return run

    depth = 2
    pend = []  # (pair_idx, exp_tile) awaiting attnV
    fillers = (vproj_filler(0), vproj_filler(1), vproj_filler(2))

    def alloc_ov(pc):
        return [ov_psum.tile([DHEAD + 1, 512], F32, name=f"ov{2 * pc + h}",
                             tag="ov", bufs=2) for h in range(2)]

    def av_mm(pc, e_t, ov, m):
        for h in range(2):
            nc.tensor.matmul(ov[h][:], vt_head_ap(2 * m, 2 * pc + h),
                             e_t[:, 2 * m:2 * m + 2, h * 512:(h + 1) * 512],
                             start=(m == 0), stop=(m == 3), perf_mode=DR)

    def finish_pair(pc, ov):
        attn_finish(2 * pc, ov[0], ov_psum, O1all[0:DHEAD, pc, :])
        attn_finish(2 * pc + 1, ov[1], ov_psum, O1all[DHEAD:2 * DHEAD, pc, :])

    for c in range(KC):
        drain = pend.pop(0) if len(pend) >= depth else None
        dov = alloc_ov(drain[0]) if drain else None
        e_t = epool.tile([P, 8, TKV], F8, name=f"exp{c}", tag="exp")
        for k8 in range(8):
            sps = sc_psum.tile([P, 1024], F32, name=f"sps{c}_{k8}", tag="sc", bufs=2)
            for h in range(2):
                nc.tensor.matmul(sps[:, h * 512:(h + 1) * 512],
                                 Kt[c][64 * h:64 * h + 64, k8 * P:(k8 + 1) * P],
                                 Qt[c][64 * h:64 * h + 64, :],
                                 start=True, stop=True, tile_position=(64 * h, 0))
            nc.scalar.activation(e_t[:, k8, :], sps[:], AF.Exp,
                                 scale=SCALE / (WS * WS), bias=ebias[:])
            if drain is not None and k8 < 4:
                av_mm(drain[0], drain[1], dov, k8)
        if drain is not None:
            finish_pair(drain[0], dov)
        if c < len(fillers):
            fillers[c]()
        pend.append((c, e_t))
    for pc, e_t in pend:
        ov = alloc_ov(pc)
        for m in range(4):
            av_mm(pc, e_t, ov, m)
        finish_pair(pc, ov)

    close("epool", "qkv", "vp_psum", "ov_psum", "sc_psum", "wpool1", "ln1p")

    # ============ phase B: O1 proj + LN2 + attn2 projections + attn2 ========

    o2p = pool("o2p", 1)
    resp = pool("resp", 1)
    wpool = pool("wpoolB", 1)
    ln2p = pool("ln2p", 1)
    psumB = pool("psumB", 1, space="PSUM")
    x1p = pool("x1p", 1, side="right")
    qkv2 = pool("qkv2", 1, side="right")

    x1 = [x1p.tile([P, T], F32, name=f"x1_{mc}", tag="x1", bufs=KC) for mc in range(KC)]
    resid = []
    for c in range(KC):
        rc = resp.tile([P, T], F32, name=f"res_{c}", tag="res", bufs=KC)
        nc.sync.dma_start(rc[:], d["xres"][c * P:(c + 1) * P, :])
        resid.append(rc)

    # context, fp8, padded to 80 tokens for the 16B dual-fp8 pair stride
    ctx_all = qkv2.tile([P, KCX, 80], F8, name="ctx_all")
    for c in range(KCX):
        cc = qkv2.tile([P, MCTX], F32, name=f"ctx_{c}", tag="ctx", bufs=2)
        nc.sync.dma_start(cc[:], d["ctxt"][c * P:(c + 1) * P, :])
        nc.vector.tensor_copy(out=ctx_all[:, c, 0:MCTX], in_=cc[:])

    ln2_all = ln2p.tile([P, KC, T], F8, name="ln2_all")
    st2 = ln8_begin(1, 0, psumB)

    def o1_consume(mc, ps):
        if trivial_bias:
            nc.vector.scalar_tensor_tensor(x1[mc][:], ps[:], 1.0 / WS, resid[mc][:],
                                           mybir.AluOpType.mult, mybir.AluOpType.add)
        else:
            t = tmp.tile([P, T], F32, name=f"o1b_{mc}", tag="o1b", bufs=2)
            nc.scalar.activation(t[:], ps[:], AF.Copy, scale=1.0 / WS, bias=bias_ap(mc))
            nc.vector.tensor_add(x1[mc][:], t[:], resid[mc][:])

    def o1_after(mc):
        if mc % 2 == 1:
            ln8_chunk_pair(st2, mc // 2, lambda c: x1[c][:])

    project8(d["wo1"], KP, lambda p8: O1all[:, 2 * p8:2 * p8 + 2, :], KC,
             o1_consume, wpool, "wo1", psumB, after=o1_after, pbufs=2)
    ln8_finish(st2, ln2_all, slice(0, 512), psumB)

    Q2t = [qkv2.tile([P, T], F8, name=f"q2t_{mc}", tag="q2t", bufs=KC) for mc in range(KC)]
    K2t = [qkv2.tile([P, MCTX], F8, name=f"k2t_{mc}", tag="k2t", bufs=KC) for mc in range(KC)]
    V2t = qkv2.tile([P, HEADS, DHEAD + 1], F8, name="v2t")
    O2all = o2p.tile([P, KC, T], F8, name="o2all")

    def q2_consume(mc, ps):
        nc.vector.tensor_copy(out=Q2t[mc][:], in_=ps[:])

    project8(d["wq2"], KP, lambda p8: ln2_all[:, 2 * p8:2 * p8 + 2, :], KC,
             q2_consume, wpool, "wq2", psumB, pbufs=2)

    for mc in range(KC):
        wt = wpool.tile([P, KPX, 2, P], F8, name=f"wk2_{mc}", tag="wk2", bufs=3)
        nc.sync.dma_start(wt[:], d["wk2"][mc])
        ps = psumB.tile([P, MCTX], F32, name=f"psk2_{mc}", tag="projx", bufs=2)
        for p8 in range(KPX):
            nc.tensor.matmul(ps[:], wt[:, p8], ctx_all[:, 2 * p8:2 * p8 + 2, 0:MCTX],
                             start=(p8 == 0), stop=(p8 == KPX - 1), perf_mode=DR)
        nc.vector.tensor_copy(out=K2t[mc][:], in_=ps[:])

    wv2_sb = []
    for p8 in range(KPX):
        wv = wpool.tile([P, 2, DIM], F8, name=f"wv2_{p8}", tag="wv2", bufs=KPX)
        nc.sync.dma_start(wv[:], d["wv2"][p8])
        wv2_sb.append(wv)

    close("psumB")

    # ---------------- attn2 ----------------

    sc_psum = pool("sc_psum2", 1, space="PSUM")
    ov_psum = pool("ov_psum2", 1, space="PSUM")
    vp_psum = pool("vp_psum2", 1, space="PSUM")
    epool = pool("epool2", 6, side="right")

    def v2proj_filler():
        nc.vector.memset(V2t[:], WS)
        for n0, nsz in ((0, 512), (512, 512), (1024, 256)):
            ps = vp_psum.tile([MCTX, 512], F32, name=f"psv2_{n0}", tag="vproj", bufs=2)
            for p8 in range(KPX):
                nc.tensor.matmul(ps[:, :nsz], ctx_all[:, 2 * p8:2 * p8 + 2, 0:MCTX],
                                 wv2_sb[p8][:, :, n0:n0 + nsz],
                                 start=(p8 == 0), stop=(p8 == KPX - 1), perf_mode=DR)
            nc.vector.tensor_copy(
                out=V2t[:MCTX, n0 // DHEAD:(n0 + nsz) // DHEAD, 0:DHEAD],
                in_=ps[:, :nsz].rearrange("p (h e) -> p h e", e=DHEAD))

    def av2(pc, e_t, tagsuf):
        ov = [ov_psum.tile([DHEAD + 1, 512], F32, name=f"ov2{tagsuf}_{2 * pc + h}",
                           tag="ov", bufs=2) for h in range(2)]
        for h in range(2):
            nc.tensor.matmul(ov[h][:], V2t[:MCTX, 2 * pc + h, :],
                             e_t[:MCTX, h * 512:(h + 1) * 512],
                             start=True, stop=True)
        attn_finish(40 + 2 * pc, ov[0], ov_psum, O2all[0:DHEAD, pc, :])
        attn_finish(41 + 2 * pc, ov[1], ov_psum, O2all[DHEAD:2 * DHEAD, pc, :])

    pend2 = []
    for c in range(KC):
        drain = pend2.pop(0) if len(pend2) >= 2 else None
        if drain is not None:
            av2(drain[0], drain[1], "d")
        sps = sc_psum.tile([MCTX, 1024], F32, name=f"sps2_{c}", tag="sc", bufs=2)
        for h in range(2):
            nc.tensor.matmul(sps[:, h * 512:(h + 1) * 512],
                             K2t[c][64 * h:64 * h + 64, :],
                             Q2t[c][64 * h:64 * h + 64, :],
                             start=True, stop=True, tile_position=(64 * h, 0))
        e_t = epool.tile([MCTX, 1024], F8, name=f"exp2_{c}", tag="exp2")
        nc.scalar.activation(e_t[:], sps[:], AF.Exp, scale=SCALE / (WS * WS),
                             bias=ebias[:MCTX])
        if c == 0:
            v2proj_filler()
        pend2.append((c, e_t))
    for pc, e_t in pend2:
        av2(pc, e_t, "t")

    close("epool2", "qkv2", "vp_psum2", "ov_psum2", "sc_psum2", "ln2p", "wpoolB",
          "resp")

    # ============ phase C: O2 proj + LN3 + GEGLU FF (fp16) ================

    wpool3 = pool("wpool3", 1)
    x2p = pool("x2p", 1)
    hhp = pool("hhp", 1)
    wpool4b = pool("wpool4b", 1)
    wpool4a = pool("wpool4a", 1)
    ln3p = pool("ln3p", 1)
    psumC = pool("psumC", 1, space="PSUM")

    # prefetch the first FF weights while O2 proj / LN3 still run
    wff2_pre = []
    for mc in range(1):
        wt = wpool4b.tile([P, JFF, P], F16, name=f"wff2_{mc}", tag="wff2", bufs=2)
        nc.sync.dma_start(wt[:], d["wff2"][mc])
        wff2_pre.append(wt)
    wg0 = wpool4a.tile([P, KC, P], F16, name="wg_0", tag="wff1g", bufs=3)
    nc.sync.dma_start(wg0[:], d["wff1"][JFF])
    wa0 = wpool4a.tile([P, KC, P], F16, name="wa_0", tag="wff1a", bufs=3)
    nc.sync.dma_start(wa0[:], d["wff1"][0])

    x2 = [x2p.tile([P, T], F32, name=f"x2_{mc}", tag="x2", bufs=KC) for mc in range(KC)]
    hht = [hhp.tile([P, T], F16, name=f"hh_{j}", tag="hh", bufs=JFF) for j in range(JFF)]
    ln3t = [ln3p.tile([P, T], F16, name=f"ln3_{c}", tag="ln3", bufs=KC) for c in range(KC)]

    st3 = ln16_begin(2, psumC)

    def o2_consume(mc, ps):
        if trivial_bias:
            nc.vector.scalar_tensor_tensor(x2[mc][:], ps[:], 1.0 / WS, x1[mc][:],
                                           mybir.AluOpType.mult, mybir.AluOpType.add)
        else:
            t = tmp.tile([P, T], F32, name=f"o2b_{mc}", tag="o1b", bufs=2)
            nc.scalar.activation(t[:], ps[:], AF.Copy, scale=1.0 / WS, bias=bias_ap(10 + mc))
            nc.vector.tensor_add(x2[mc][:], t[:], x1[mc][:])

    def o2_after(mc):
        ln16_chunk(st3, mc, x2[mc][:])

    project8(d["wo2"], KP, lambda p8: O2all[:, 2 * p8:2 * p8 + 2, :], KC,
             o2_consume, wpool3, "wo2", psumC, after=o2_after, pbufs=2)
    ln16_finish(st3, ln3t, psumC)
    close("x1p", "psumC")
    psumFF = pool("psumFF", 1, space="PSUM")

    for j in range(JFF):
        if j == 0:
            wg, wa = wg0, wa0
        else:
            wg = wpool4a.tile([P, KC, P], F16, name=f"wg_{j}", tag="wff1g", bufs=3)
            nc.sync.dma_start(wg[:], d["wff1"][JFF + j])
            wa = wpool4a.tile([P, KC, P], F16, name=f"wa_{j}", tag="wff1a", bufs=3)
            nc.sync.dma_start(wa[:], d["wff1"][j])
        gps = psumFF.tile([P, 512], F32, name=f"gps_{j}", tag="proj", bufs=4)
        for kc in range(KC):
            nc.tensor.matmul(gps[:], wg[:, kc], ln3t[kc][:], start=(kc == 0),
                             stop=(kc == KC - 1))
        gel = tmp.tile([P, T], F16, name=f"gel_{j}", tag="gel", bufs=3)
        if trivial_bias:
            nc.scalar.activation(gel[:], gps[:], AF.Gelu_apprx_tanh)
        else:
            nc.scalar.activation(gel[:], gps[:], AF.Gelu_apprx_tanh, bias=bias_ap(60 + j))
        aps = psumFF.tile([P, 512], F32, name=f"aps_{j}", tag="proj", bufs=4)
        for kc in range(KC):
            nc.tensor.matmul(aps[:], wa[:, kc], ln3t[kc][:], start=(kc == 0),
                             stop=(kc == KC - 1))
        if trivial_bias:
            nc.vector.tensor_mul(hht[j][:], aps[:], gel[:])
        else:
            nc.vector.scalar_tensor_tensor(hht[j][:], aps[:], bias_ap(20 + j), gel[:],
                                           mybir.AluOpType.add, mybir.AluOpType.mult)

    close("ln3p", "wpool4a")

    # ---------------- FF down-proj + residual -> out ----------------

    outp = pool("outp", 4)
    for mc in range(KC):
        if mc < 1:
            wt = wff2_pre[mc]
        else:
            wt = wpool4b.tile([P, JFF, P], F16, name=f"wff2_{mc}", tag="wff2", bufs=2)
            nc.sync.dma_start(wt[:], d["wff2"][mc])
        ps = psumFF.tile([P, 512], F32, name=f"psf2_{mc}", tag="proj", bufs=4)
        for kc in range(JFF):
            nc.tensor.matmul(ps[:], wt[:, kc], hht[kc][:], start=(kc == 0),
                             stop=(kc == JFF - 1))
        ot = outp.tile([P, T], F32, name=f"out_{mc}", tag="out")
        if trivial_bias:
            nc.vector.tensor_add(ot[:], ps[:], x2[mc][:])
        else:
            nc.vector.scalar_tensor_tensor(ot[:], ps[:], bias_ap(100 + mc), x2[mc][:],
                                           mybir.AluOpType.add, mybir.AluOpType.add)
        nc.sync.dma_start(d["out"][mc * P:(mc + 1) * P, :], ot[:])

    close("outp", "wpool4b", "hhp", "x2p", "wpool3", "o2p", "otp", "tmp", "const", "psumFF")


def _lhst_layout(w, n_kc, n_mc):
    """[K, M] f32 -> fp16 [n_mc, 128, n_kc, 128] so block [mc] is the
    contiguous stationary-operand group for output chunk mc."""
    return np.ascontiguousarray(
        w.reshape(n_kc, P, n_mc, P).transpose(2, 1, 0, 3).astype(np.float16))


def _lhst8_layout(w, n_kc, n_mc):
    """[K, M] f32 -> fp8 x16 [n_mc, 128, n_kc//2, 2, 128]: dual-fp8 stationary
    pairs over adjacent k-chunks."""
    a = (np.asarray(w, np.float32) * WS).reshape(n_kc // 2, 2, P, n_mc, P)
    return np.ascontiguousarray(a.transpose(3, 2, 0, 1, 4)).astype(E4)


def _rhs8_layout(w, n_kc):
    """[K, M] f32 -> fp8 x16 [n_kc//2, 128, 2, M]: dual-fp8 moving pairs."""
    a = (np.asarray(w, np.float32) * WS).reshape(n_kc // 2, 2, P, -1)
    return np.ascontiguousarray(a.transpose(0, 2, 1, 3)).astype(E4)


_BUILT = {}


def _build(trivial_aff, trivial_bias):
    key = (trivial_aff, trivial_bias)
    if key in _BUILT:
        return _BUILT[key]
    nc = bacc.Bacc("TRN2", target_bir_lowering=False, debug=False, num_devices=N_CORES)
    d = {
        "xt": nc.dram_tensor("xt", [DIM, TKV], F32, kind="ExternalInput").ap(),
        "ctxt": nc.dram_tensor("ctxt", [CTX_DIM, MCTX], F32, kind="ExternalInput").ap(),
        "xres": nc.dram_tensor("xres", [DIM, T], F32, kind="ExternalInput").ap(),
        "wq1": nc.dram_tensor("wq1", [KC, P, KP, 2, P], F8, kind="ExternalInput").ap(),
        "wk1": nc.dram_tensor("wk1", [KC, P, KP, 2, P], F8, kind="ExternalInput").ap(),
        "wv1": nc.dram_tensor("wv1", [KP, P, 2, DIM], F8, kind="ExternalInput").ap(),
        "wo1": nc.dram_tensor("wo1", [KC, P, KP, 2, P], F8, kind="ExternalInput").ap(),
        "wq2": nc.dram_tensor("wq2", [KC, P, KP, 2, P], F8, kind="ExternalInput").ap(),
        "wk2": nc.dram_tensor("wk2", [KC, P, KPX, 2, P], F8, kind="ExternalInput").ap(),
        "wv2": nc.dram_tensor("wv2", [KPX, P, 2, DIM], F8, kind="ExternalInput").ap(),
        "wo2": nc.dram_tensor("wo2", [KC, P, KP, 2, P], F8, kind="ExternalInput").ap(),
        "wff1": nc.dram_tensor("wff1", [2 * JFF, P, KC, P], F16, kind="ExternalInput").ap(),
        "wff2": nc.dram_tensor("wff2", [KC, P, JFF, P], F16, kind="ExternalInput").ap(),
        "out": nc.dram_tensor("out", [DIM, T], F32, kind="ExternalOutput").ap(),
    }
    if not trivial_aff:
        d["aff"] = nc.dram_tensor("aff", [P, 60], F32, kind="ExternalInput").ap()
    if not trivial_bias:
        d["biases"] = nc.dram_tensor("biases", [P, 110], F32, kind="ExternalInput").ap()
    with tile.TileContext(nc) as tc:
        _emit(tc, d, trivial_aff, trivial_bias)
    nc.compile()
    _BUILT[key] = nc
    return nc


def kernel(x, context,
           g1, be1, wq1, wk1, wv1, wo1, bo1,
           g2, be2, wq2, wk2, wv2, wo2, bo2,
           g3, be3, w_ff1, b_ff1, w_ff2, b_ff2,
           _trace=False):
    global last_exec_time_ns
    x = np.asarray(x, np.float32)
    context = np.asarray(context, np.float32)

    affs = [np.asarray(a, np.float32) for a in (g1, be1, g2, be2, g3, be3)]
    biases = [np.asarray(b, np.float32) for b in (bo1, bo2, b_ff1, b_ff2)]
    trivial_aff = all(np.all(a == (1.0 if i % 2 == 0 else 0.0))
                      for i, a in enumerate(affs))
    trivial_bias = all(np.all(b == 0.0) for b in biases)

    nc = _build(trivial_aff, trivial_bias)

    shared = {
        "wq1": _lhst8_layout(np.asarray(wq1, np.float32), KC, KC),
        "wk1": _lhst8_layout(np.asarray(wk1, np.float32), KC, KC),
        "wv1": _rhs8_layout(np.asarray(wv1, np.float32), KC),
        "wo1": _lhst8_layout(np.asarray(wo1, np.float32), KC, KC),
        "wq2": _lhst8_layout(np.asarray(wq2, np.float32), KC, KC),
        "wk2": _lhst8_layout(np.asarray(wk2, np.float32), KCX, KC),
        "wv2": _rhs8_layout(np.asarray(wv2, np.float32), KCX),
        "wo2": _lhst8_layout(np.asarray(wo2, np.float32), KC, KC),
        "wff1": _lhst_layout(np.asarray(w_ff1, np.float32), KC, 2 * JFF),
        "wff2": _lhst_layout(np.asarray(w_ff2, np.float32), JFF, KC),
    }
    if not trivial_aff:
        aff = np.zeros([P, 60], np.float32)
        for i, a in enumerate(affs):
            # col = ln_idx*20 + (0 for g / 10 for be) + chunk
            ln_idx, j = i // 2, i % 2
            aff[:, ln_idx * 20 + j * 10: ln_idx * 20 + j * 10 + 10] = \
                a.reshape(KC, P).T
        shared["aff"] = aff
    if not trivial_bias:
        bb = np.zeros([P, 110], np.float32)
        bb[:, 0:10] = biases[0].reshape(KC, P).T
        bb[:, 10:20] = biases[1].reshape(KC, P).T
        bb[:, 20:100] = biases[2].reshape(2 * JFF, P).T
        bb[:, 100:110] = biases[3].reshape(KC, P).T
        shared["biases"] = bb

    in_maps = []
    for b in range(BATCH):
        ctxt = np.ascontiguousarray(context[b].T)
        for h in range(2):
            xr = np.roll(x[b], -h * T, axis=0)
            m = dict(shared)
            xrt = np.ascontiguousarray(xr.T)
            m["xt"] = xrt
            m["xres"] = np.ascontiguousarray(xrt[:, 0:T])
            m["ctxt"] = ctxt
            in_maps.append(m)

    res = bass_utils.run_bass_kernel_spmd(
        nc, in_maps, core_ids=list(range(N_CORES)), trace=_trace)
    last_exec_time_ns = res.exec_time_ns

    out = np.empty((BATCH, NTOK, DIM), np.float32)
    for b in range(BATCH):
        for h in range(2):
            out[b, h * T:(h + 1) * T, :] = res.results[b * 2 + h]["out"].T
    return out
